# revision 1
# baseline (speedup 1.0000x reference)
"""Trainium2 Bass kernel for nn_ActorNetwork (2-layer GCN + actor head).

Self-contained: hardcodes all shapes/sharding (8 NeuronCores).

Strategy:
  - Shard dst nodes (= graphs) contiguously across 8 cores (10240 nodes =
    256 graphs per core).
  - Edges are random over the full node set; each layer gathers source rows
    with gpsimd dma_gather (edges sorted by (src-chunk, dst-tile) on host),
    aggregates per 128-dst tile with one-hot matmuls on TensorE (bf16).
  - Self-loops bypass the gather (sequential stream + PE transpose).
  - Between layers: AllGather of the dinv-prescaled h1@W2 ("m2s") so every
    core can gather any source row of layer 2.
  - Head: host rows are static (first 13 of each 40); strided SBUF->SBUF
    DMAs build the [13*64, graphs] lhsT; dst-side dinv/bias/relu applied on
    the selected slots only; f32 GEMM + softmax.
"""
import sys
import hashlib

sys.path.insert(0, "/opt/trn_rl_repo")

import numpy as np
import ml_dtypes
from contextlib import ExitStack

from concourse import bass, mybir, tile, bass_utils, bacc
from concourse.masks import make_identity

F32 = mybir.dt.float32
BF16 = mybir.dt.bfloat16
I16 = mybir.dt.int16
I32 = mybir.dt.int32

N_CORES = 8
N = 81920
NL = N // N_CORES          # 10240 nodes per core
IN_DIM = 128
H1 = 256
H2 = 64
GRAPH = 40
NH = 13
ACT = 145
TILES = NL // 128          # 80 dst tiles per core
GPC = NL // GRAPH          # 256 graphs per core
CHUNK = 32768
CHUNKS = [(0, 32768), (32768, 32768), (65536, 16384)]
NCH = 3
CALL_G = 16                # groups (of 128 idxs) per dma_gather call
SENT = 300.0               # sentinel dst value for padding slots


# ---------------------------------------------------------------- host prep

def _prep(ei):
    src = ei[0].astype(np.int64)
    dst = ei[1].astype(np.int64)
    deg = np.bincount(dst, minlength=N).astype(np.float64) + 1.0
    dinv = (1.0 / np.sqrt(deg)).astype(np.float32)
    coef = (dinv[src] * dinv[dst]).astype(np.float32)

    core = dst // NL
    t_of = (dst % NL) // 128
    c_of = src // CHUNK
    idxl = src % CHUNK
    dloc = (dst % 128).astype(np.float32)

    counts = np.zeros((N_CORES, NCH, TILES), np.int64)
    np.add.at(counts, (core, c_of, t_of), 1)
    Ncm = counts.max(axis=0)                        # [3, 80] common counts
    seg_off = np.zeros((NCH, TILES), np.int64)      # global slot offsets
    chunk_base = np.zeros(NCH + 1, np.int64)
    calls = []   # (chunk, slot0_global, n_g, events)
    off = 0
    for c in range(NCH):
        chunk_base[c] = off
        for t in range(TILES):
            seg_off[c, t] = off
            off += int(Ncm[c, t])
        off = ((off - chunk_base[c] + 127) // 128 + 0) * 128 + chunk_base[c] \
            if (off - chunk_base[c]) % 128 else off
    chunk_base[NCH] = off
    L = int(off)

    # per-chunk group structure + matmul/drain events
    for c in range(NCH):
        base = int(chunk_base[c])
        S = int(chunk_base[c + 1] - base)
        ngroups = S // 128
        # tile of each group's first slot
        def tile_of(slot):
            # slot is chunk-local
            j = np.searchsorted(seg_off[c] - base, slot, side="right") - 1
            j = max(0, min(TILES - 1, int(j)))
            if slot >= int(seg_off[c, j] - base) + int(Ncm[c, j]):
                return -1          # chunk-tail pad region
            return j
        tg = [tile_of(128 * g) for g in range(ngroups)]
        for g in range(ngroups):
            if tg[g] == -1:
                tg[g] = TILES - 1  # tail pads: harmless window
        # first/last group of each tile's segment
        g_a = [(int(seg_off[c, t] - base)) // 128 for t in range(TILES)]
        g_b = [(int(seg_off[c, t] - base) + int(Ncm[c, t]) - 1) // 128
               for t in range(TILES)]
        # build matmul event list in group order
        events_all = []
        for g in range(ngroups):
            t0 = tg[g]
            seg_end = int(seg_off[c, t0] - base) + int(Ncm[c, t0])
            spans = (t0 + 1 < TILES) and (128 * (g + 1) > seg_end)
            if spans:
                assert 128 * (g + 1) <= seg_end + int(Ncm[c, t0 + 1]), \
                    "group spans >2 tiles"
            evs = [(g, 0, t0, g == g_a[t0], g == g_b[t0])]
            if spans:
                t1 = t0 + 1
                evs.append((g, 1, t1, g == g_a[t1], g == g_b[t1]))
            events_all.append(evs)
        # slice into calls
        gi = 0
        while gi < ngroups:
            n = min(CALL_G, ngroups - gi)
            evs = []
            for g in range(gi, gi + n):
                for (gg, half, t, st, sp) in events_all[g]:
                    evs.append((gg - gi, half, t, st, sp))
            calls.append((c, base + 128 * gi, n, evs))
            gi += n

    idx_all = np.zeros((N_CORES, L), np.int16)
    dstv_all = np.full((N_CORES, L), SENT, np.float32)
    coef_all = np.zeros((N_CORES, L), np.float32)
    # group tile map per global slot (for relative dstv)
    tg_of_slot = np.full(L, -1, np.int64)
    for c in range(NCH):
        base = int(chunk_base[c])
        S = int(chunk_base[c + 1] - base)
        for g in range(S // 128):
            j = np.searchsorted(seg_off[c] - base, 128 * g, side="right") - 1
            j = max(0, min(TILES - 1, int(j)))
            if 128 * g >= int(seg_off[c, j] - base) + int(Ncm[c, j]):
                j = TILES - 1
            tg_of_slot[base + 128 * g: base + 128 * (g + 1)] = j
    for r in range(N_CORES):
        m = core == r
        sc, st = c_of[m], t_of[m]
        si, sd, scf = idxl[m], dloc[m], coef[m]
        order = np.lexsort((st, sc))
        sc, st = sc[order], st[order]
        si, sd, scf = si[order], sd[order], scf[order]
        key = sc * TILES + st
        change = np.r_[True, key[1:] != key[:-1]]
        starts = np.flatnonzero(change)
        runid = np.cumsum(change) - 1
        within = np.arange(len(key)) - starts[runid]
        base_run = seg_off[sc[starts], st[starts]]
        pos = base_run[runid] + within
        idx_all[r, pos] = si.astype(np.int16)
        dstv_all[r, pos] = sd + 128.0 * (st - tg_of_slot[pos])
        coef_all[r, pos] = scf

    idx_sb = np.stack([
        np.tile(idx_all[r].reshape(-1, 16).T, (8, 1)) for r in range(N_CORES)
    ])                                               # [8, 128, L/16]
    dstv_sb = np.stack([
        dstv_all[r].reshape(-1, 128).T for r in range(N_CORES)
    ]).astype(ml_dtypes.bfloat16)                    # [8, 128, L/128]
    coef_sb = np.stack([
        coef_all[r].reshape(-1, 128).T for r in range(N_CORES)
    ])                                               # [8, 128, L/128]

    dinv_l = dinv.reshape(N_CORES, NL)
    dinv_tiles = np.ascontiguousarray(
        dinv_l.reshape(N_CORES, TILES, 128).transpose(0, 2, 1))   # [8,128,80]
    dinv2_tiles = (dinv_tiles ** 2).astype(np.float32)

    # per-slot dst dinv for the head: hzT[p, k, g] -> host h=2k+(p>=64),
    # feat=p%64, local node g*40+h
    dinv_hz = np.zeros((N_CORES, 128, 7, GPC), np.float32)
    for k in range(7):
        for half in range(2):
            h = 2 * k + half
            if h >= NH:
                continue
            nodes = np.arange(GPC) * GRAPH + h
            dinv_hz[:, 64 * half:64 * (half + 1), k, :] = \
                dinv_l[:, nodes][:, None, :]

    return dict(L=L, calls=calls, idx_sb=idx_sb, dstv_sb=dstv_sb,
                coef_sb=coef_sb, dinv_tiles=dinv_tiles,
                dinv2_tiles=dinv2_tiles, dinv_hz=dinv_hz)


# ---------------------------------------------------------------- builder

def _edge_pass(nc, wk, psA, calls, src_dram, elem, idxt, dstvt, coeft,
               iota_bf, agg, selfT, l2):
    """Shared edge-aggregation pass for both layers (256-dst windows)."""
    open_ps = {}
    for (c, slot0, n_g, events) in calls:
        rows0, nrows = CHUNKS[c]
        gat = wk.tile([128, CALL_G, elem], F32, tag="gat", bufs=3)
        nc.gpsimd.dma_gather(
            out_ap=gat[:, 0:n_g, :],
            in_ap=src_dram[rows0:rows0 + nrows, :],
            idxs_ap=idxt[:, slot0 // 16: slot0 // 16 + n_g * 8],
            num_idxs=n_g * 128, num_idxs_reg=n_g * 128,
            elem_size=elem, single_packet=False)
        s0 = slot0 // 128
        gatb = wk.tile([128, CALL_G, elem], BF16, tag="gatb", bufs=3)
        if l2:
            nc.vector.tensor_copy(out=gatb[:, 0:n_g, :], in_=gat[:, 0:n_g, :])
        else:
            nc.vector.tensor_tensor(
                out=gatb[:, 0:n_g, :], in0=gat[:, 0:n_g, :],
                in1=coeft[:, s0:s0 + n_g].unsqueeze(2).to_broadcast(
                    (128, n_g, elem)),
                op=mybir.AluOpType.mult)
        oh = wk.tile([128, CALL_G, 256], BF16, tag="oh", bufs=3)
        nc.vector.tensor_tensor(
            out=oh[:, 0:n_g, :],
            in0=dstvt[:, s0:s0 + n_g].unsqueeze(2).to_broadcast((128, n_g, 256)),
            in1=iota_bf[:].unsqueeze(1).to_broadcast((128, n_g, 256)),
            op=mybir.AluOpType.is_equal)
        M = 64 if l2 else 128
        for (g, half, t, first, last) in events:
            if first:
                open_ps[t] = psA.tile([M, 128], F32, tag="agg",
                                      name=f"aggps_c{c}_t{t}")
            ps = open_ps[t]
            nc.tensor.matmul(out=ps[:], lhsT=gatb[:, g, :],
                             rhs=oh[:, g, 128 * half:128 * (half + 1)],
                             start=first, stop=last)
            if last:
                sl = slice(128 * t, 128 * (t + 1))
                if c == 0:
                    nc.vector.tensor_tensor(out=agg[:, sl], in0=ps[:],
                                            in1=selfT[:, sl],
                                            op=mybir.AluOpType.add)
                else:
                    nc.vector.tensor_tensor(out=agg[:, sl], in0=agg[:, sl],
                                            in1=ps[:], op=mybir.AluOpType.add)
                del open_ps[t]


def _build(L, calls):
    nc = bacc.Bacc("TRN2", target_bir_lowering=False, debug=False,
                   num_devices=N_CORES)
    d_xfull = nc.dram_tensor("xfull", [N, IN_DIM], F32, kind="ExternalInput")
    d_xloc = nc.dram_tensor("xloc", [NL, IN_DIM], F32, kind="ExternalInput")
    d_idx = nc.dram_tensor("idx", [128, L // 16], I16, kind="ExternalInput")
    d_dstv = nc.dram_tensor("dstv", [128, L // 128], BF16, kind="ExternalInput")
    d_coef = nc.dram_tensor("coef", [128, L // 128], F32, kind="ExternalInput")
    d_dinvt = nc.dram_tensor("dinvt", [128, TILES], F32, kind="ExternalInput")
    d_dinv2t = nc.dram_tensor("dinv2t", [128, TILES], F32, kind="ExternalInput")
    d_dinvhz = nc.dram_tensor("dinvhz", [128, 7 * GPC], F32, kind="ExternalInput")
    d_W1 = nc.dram_tensor("W1", [IN_DIM, H1], F32, kind="ExternalInput")
    d_b1 = nc.dram_tensor("b1", [H1, 1], F32, kind="ExternalInput")
    d_W2 = nc.dram_tensor("W2", [H1, H2], F32, kind="ExternalInput")
    d_b2hz = nc.dram_tensor("b2hz", [128, 1], F32, kind="ExternalInput")
    d_Wout = nc.dram_tensor("Wout", [NH * H2, ACT], F32, kind="ExternalInput")
    d_bout = nc.dram_tensor("bout", [1, ACT], F32, kind="ExternalInput")
    d_out = nc.dram_tensor("out", [GPC, ACT], F32, kind="ExternalOutput")

    with tile.TileContext(nc) as tc, ExitStack() as top:
        perm = top.enter_context(tc.tile_pool(name="perm", bufs=1))
        dram = top.enter_context(tc.tile_pool(name="dram", bufs=1, space="DRAM"))

        # ---- persistent tiles
        idxt = perm.tile([128, L // 16], I16)
        nc.sync.dma_start(out=idxt[:], in_=d_idx[:])
        dstvt = perm.tile([128, L // 128], BF16)
        nc.sync.dma_start(out=dstvt[:], in_=d_dstv[:])
        coeft = perm.tile([128, L // 128], F32)
        nc.sync.dma_start(out=coeft[:], in_=d_coef[:])
        dinvt = perm.tile([128, TILES], F32)
        nc.sync.dma_start(out=dinvt[:], in_=d_dinvt[:])
        dinv2t = perm.tile([128, TILES], F32)
        nc.sync.dma_start(out=dinv2t[:], in_=d_dinv2t[:])
        W1sb = perm.tile([128, H1], BF16)
        nc.gpsimd.dma_start(out=W1sb[:], in_=d_W1[:])
        b1sb = perm.tile([128, 2], F32)
        nc.sync.dma_start(out=b1sb[:, 0:1], in_=d_b1[0:128, :])
        nc.sync.dma_start(out=b1sb[:, 1:2], in_=d_b1[128:256, :])
        W2sb = perm.tile([128, 2, H2], BF16)
        nc.gpsimd.dma_start(out=W2sb[:, 0, :], in_=d_W2[0:128, :])
        nc.gpsimd.dma_start(out=W2sb[:, 1, :], in_=d_W2[128:256, :])
        b2hz = perm.tile([128, 1], F32)
        nc.sync.dma_start(out=b2hz[:], in_=d_b2hz[:])
        WoutSB = perm.tile([128, 7, ACT], F32)
        for k in range(6):
            nc.sync.dma_start(out=WoutSB[:, k, :],
                              in_=d_Wout[128 * k:128 * (k + 1), :])
        nc.sync.dma_start(out=WoutSB[0:64, 6, :], in_=d_Wout[768:832, :])
        boutrep = perm.tile([128, ACT], F32)
        nc.sync.dma_start(out=boutrep[:], in_=d_bout[:].to_broadcast((128, ACT)))
        dinvhz = perm.tile([128, 7, GPC], F32)
        nc.sync.dma_start(out=dinvhz[:].rearrange("p k g -> p (k g)"),
                          in_=d_dinvhz[:])

        ident = perm.tile([128, 128], F32)
        make_identity(nc, ident[:])
        iota_i = perm.tile([128, 256], I32)
        nc.gpsimd.iota(iota_i[:], pattern=[[1, 256]], base=0,
                       channel_multiplier=0)
        iota_bf = perm.tile([128, 256], BF16)
        nc.vector.tensor_copy(out=iota_bf[:], in_=iota_i[:])
        iota_f = perm.tile([128, 128], F32)
        nc.vector.tensor_copy(out=iota_f[:], in_=iota_i[:, 0:128])
        # ident_hi[p, j] = 1 if j == p + 64 (used to shift rows up by 64)
        ioc = perm.tile([128, 1], I32)
        nc.gpsimd.iota(ioc[:], pattern=[[1, 1]], base=64, channel_multiplier=1)
        iocf = perm.tile([128, 1], F32)
        nc.vector.tensor_copy(out=iocf[:], in_=ioc[:])
        ident_hi = perm.tile([128, 128], F32)
        nc.vector.tensor_tensor(out=ident_hi[:],
                                in0=iocf[:].to_broadcast((128, 128)),
                                in1=iota_f[:], op=mybir.AluOpType.is_equal)

        h1T = perm.tile([128, 2, NL], BF16)

        # =========================== Layer 1 ===========================
        with ExitStack() as ph1:
            mid1 = ph1.enter_context(tc.tile_pool(name="mid1", bufs=1))
            wk1 = ph1.enter_context(tc.tile_pool(name="wk1", bufs=2))
            psA = ph1.enter_context(tc.tile_pool(name="psA", bufs=4,
                                                 space="PSUM"))
            psT = ph1.enter_context(tc.tile_pool(name="psT", bufs=2,
                                                 space="PSUM"))

            agg1 = mid1.tile([128, NL], BF16)
            xTs = mid1.tile([128, NL], BF16)
            for t in range(TILES):
                xl = wk1.tile([128, 128], F32, tag="xl")
                nc.sync.dma_start(out=xl[:], in_=d_xloc[128 * t:128 * (t + 1), :])
                xls = wk1.tile([128, 128], F32, tag="xls")
                nc.vector.tensor_scalar_mul(xls[:], xl[:], dinv2t[:, t:t + 1])
                pt = psT.tile([128, 128], F32, tag="tr")
                nc.tensor.transpose(out=pt[:], in_=xls[:], identity=ident[:])
                nc.scalar.activation(out=xTs[:, 128 * t:128 * (t + 1)],
                                     in_=pt[:],
                                     func=mybir.ActivationFunctionType.Copy)

            _edge_pass(nc, wk1, psA, calls, d_xfull, IN_DIM, idxt, dstvt,
                       coeft, iota_bf, agg1, xTs, l2=False)

            with tc.tile_pool(name="psG1", bufs=2, space="PSUM") as psG:
                for m in range(2):
                    for nb in range(NL // 512):
                        pg = psG.tile([128, 512], F32, tag="g1")
                        nc.tensor.matmul(
                            out=pg[:], lhsT=W1sb[:, 128 * m:128 * (m + 1)],
                            rhs=agg1[:, 512 * nb:512 * (nb + 1)],
                            start=True, stop=True)
                        nc.scalar.activation(
                            out=h1T[:, m, 512 * nb:512 * (nb + 1)], in_=pg[:],
                            func=mybir.ActivationFunctionType.Relu,
                            bias=b1sb[:, m:m + 1], scale=1.0)

        # ================== GEMM2 + m2s + AllGather ====================
        m2sl = dram.tile([NL, H2], F32)
        m2sf = dram.tile([N, H2], F32, addr_space="Shared")
        with ExitStack() as ph2:
            midA = ph2.enter_context(tc.tile_pool(name="midA", bufs=1))
            m2sTs = midA.tile([64, NL], BF16)
            agg2 = midA.tile([64, NL], F32)
            with ExitStack() as ph2a:
                mid2 = ph2a.enter_context(tc.tile_pool(name="mid2", bufs=1))
                psG2 = ph2a.enter_context(tc.tile_pool(name="psG2", bufs=2,
                                                       space="PSUM"))
                psT2 = ph2a.enter_context(tc.tile_pool(name="psT2", bufs=2,
                                                       space="PSUM"))
                stage = mid2.tile([128, TILES, H2], F32)
                for t in range(TILES):
                    pg = psG2.tile([128, H2], F32, tag="g2")
                    for m in range(2):
                        nc.tensor.matmul(
                            out=pg[:], lhsT=h1T[:, m, 128 * t:128 * (t + 1)],
                            rhs=W2sb[:, m, :], start=(m == 0), stop=(m == 1))
                    nc.vector.tensor_scalar_mul(stage[:, t, :], pg[:],
                                                dinvt[:, t:t + 1])
                for t in range(TILES):
                    pt = psT2.tile([64, 128], F32, tag="tr2")
                    nc.tensor.transpose(out=pt[:], in_=stage[:, t, :],
                                        identity=ident[:])
                    nc.scalar.activation(
                        out=m2sTs[:, 128 * t:128 * (t + 1)], in_=pt[:],
                        func=mybir.ActivationFunctionType.Copy)
                nc.sync.dma_start(
                    out=m2sl[:].rearrange("(t p) f -> p t f", p=128),
                    in_=stage[:])
            nc.gpsimd.collective_compute(
                "AllGather", mybir.AluOpType.bypass,
                replica_groups=[list(range(N_CORES))],
                ins=[m2sl[:].opt()], outs=[m2sf[:].opt()])

            # ========================= Layer 2 =========================
            with ExitStack() as ph3:
                wk2 = ph3.enter_context(tc.tile_pool(name="wk2", bufs=2))
                psA2 = ph3.enter_context(tc.tile_pool(name="psA2", bufs=4,
                                                      space="PSUM"))
                _edge_pass(nc, wk2, psA2, calls, m2sf, H2, idxt, dstvt,
                           coeft, iota_bf, agg2, m2sTs, l2=True)

            # ===================== actor head ==========================
            with ExitStack() as ph4:
                mid4 = ph4.enter_context(tc.tile_pool(name="mid4", bufs=1))
                wk4 = ph4.enter_context(tc.tile_pool(name="wk4", bufs=2))
                psF = ph4.enter_context(tc.tile_pool(name="psF", bufs=2,
                                                     space="PSUM"))
                hzT = mid4.tile([128, 7, GPC], F32)
                h2r = agg2[:].rearrange("p (g q) -> p q g", q=GRAPH)
                for k in range(7):
                    pk = psF.tile([128, GPC], F32, tag="hz", name=f"hzps{k}")
                    nc.tensor.matmul(out=pk[:], lhsT=ident[0:64, :],
                                     rhs=h2r[:, 2 * k, :],
                                     start=True, stop=(k == 6))
                    if k < 6:
                        nc.tensor.matmul(out=pk[:], lhsT=ident_hi[0:64, :],
                                         rhs=h2r[:, 2 * k + 1, :],
                                         start=False, stop=True)
                    nc.vector.tensor_tensor(out=hzT[:, k, :], in0=pk[:],
                                            in1=dinvhz[:, k, :],
                                            op=mybir.AluOpType.mult)
                nc.scalar.activation(out=hzT[:].rearrange("p k g -> p (k g)"),
                                     in_=hzT[:].rearrange("p k g -> p (k g)"),
                                     func=mybir.ActivationFunctionType.Relu,
                                     bias=b2hz[:, 0:1], scale=1.0)
                for m in range(GPC // 128):
                    pf = psF.tile([128, ACT], F32, tag="fin")
                    for k in range(6):
                        nc.tensor.matmul(
                            out=pf[:], lhsT=hzT[:, k, 128 * m:128 * (m + 1)],
                            rhs=WoutSB[:, k, :], start=(k == 0), stop=False)
                    nc.tensor.matmul(
                        out=pf[:], lhsT=hzT[0:64, 6, 128 * m:128 * (m + 1)],
                        rhs=WoutSB[0:64, 6, :], start=False, stop=True)
                    nc.vector.tensor_tensor(out=pf[:], in0=pf[:],
                                            in1=boutrep[:],
                                            op=mybir.AluOpType.add)
                    mx = wk4.tile([128, 1], F32, tag="mx")
                    nc.vector.tensor_reduce(out=mx[:], in_=pf[:],
                                            axis=mybir.AxisListType.X,
                                            op=mybir.AluOpType.max)
                    nmx = wk4.tile([128, 1], F32, tag="nmx")
                    nc.vector.tensor_scalar_mul(nmx[:], mx[:], -1.0)
                    esb = wk4.tile([128, ACT], F32, tag="esb")
                    nc.scalar.activation(out=esb[:], in_=pf[:],
                                         func=mybir.ActivationFunctionType.Exp,
                                         bias=nmx[:, 0:1], scale=1.0)
                    ssum = wk4.tile([128, 1], F32, tag="ssum")
                    nc.vector.tensor_reduce(out=ssum[:], in_=esb[:],
                                            axis=mybir.AxisListType.X,
                                            op=mybir.AluOpType.add)
                    rcp = wk4.tile([128, 1], F32, tag="rcp")
                    nc.vector.reciprocal(out=rcp[:], in_=ssum[:])
                    osb = wk4.tile([128, ACT], F32, tag="osb")
                    nc.vector.tensor_scalar_mul(osb[:], esb[:], rcp[:, 0:1])
                    nc.sync.dma_start(out=d_out[128 * m:128 * (m + 1), :],
                                      in_=osb[:])

    nc.compile()
    return nc


# ---------------------------------------------------------------- entry

_CACHE = {}


def _get(x, ei):
    key = hashlib.sha1(ei.tobytes()).hexdigest()
    if key not in _CACHE:
        meta = _prep(ei)
        nc = _build(meta["L"], meta["calls"])
        _CACHE[key] = (meta, nc)
    return _CACHE[key]


def _in_maps(meta, x, W1, b1, W2, b2, Wout, bout):
    b2t = np.tile(np.asarray(b2, np.float32).reshape(H2), 2).reshape(128, 1)
    maps = []
    for r in range(N_CORES):
        maps.append({
            "xfull": x,
            "xloc": np.ascontiguousarray(x[r * NL:(r + 1) * NL, :]),
            "idx": np.ascontiguousarray(meta["idx_sb"][r]),
            "dstv": np.ascontiguousarray(meta["dstv_sb"][r]),
            "coef": np.ascontiguousarray(meta["coef_sb"][r]),
            "dinvt": np.ascontiguousarray(meta["dinv_tiles"][r]),
            "dinv2t": np.ascontiguousarray(meta["dinv2_tiles"][r]),
            "dinvhz": np.ascontiguousarray(
                meta["dinv_hz"][r].reshape(128, 7 * GPC)),
            "W1": np.ascontiguousarray(W1, np.float32),
            "b1": np.ascontiguousarray(b1, np.float32).reshape(H1, 1),
            "W2": np.ascontiguousarray(W2, np.float32),
            "b2hz": b2t,
            "Wout": np.ascontiguousarray(Wout, np.float32),
            "bout": np.ascontiguousarray(bout, np.float32).reshape(1, ACT),
        })
    return maps


def kernel(x, ei, W1, b1, W2, b2, Wout, bout, _trace=False):
    x = np.ascontiguousarray(x, np.float32)
    ei = np.ascontiguousarray(ei, np.int32)
    meta, nc = _get(x, ei)
    maps = _in_maps(meta, x, W1, b1, W2, b2, Wout, bout)
    res = bass_utils.run_bass_kernel_spmd(
        nc, maps, core_ids=list(range(N_CORES)), trace=_trace)
    out = np.concatenate([res.results[r]["out"] for r in range(N_CORES)],
                         axis=0).astype(np.float32)
    if _trace:
        return out, res.exec_time_ns
    return out


def install_profile_hook():
    import types
    sys.path.insert(0, "/root/.axon_site")
    import trn_agent_boot.trn_boot as _tb
    import antenv
    if "antenv.axon_hooks" not in sys.modules:
        _mod = types.ModuleType("antenv.axon_hooks")
        _h = [None]
        _mod.set_axon_ntff_profile_hook = lambda h: _h.__setitem__(0, h)
        _mod.get_axon_ntff_profile_hook = lambda: _h[0]
        sys.modules["antenv.axon_hooks"] = _mod
        antenv.axon_hooks = _mod
        _mod.set_axon_ntff_profile_hook(
            _tb._ntff_profile_via_ctypes("/opt/axon/libaxon_pjrt.so"))



# revision 3
# speedup vs baseline: 1.3893x; 1.3893x over previous
"""Trainium2 Bass kernel for nn_ActorNetwork (2-layer GCN + actor head).

Self-contained: hardcodes all shapes/sharding (8 NeuronCores).

Strategy:
  - Shard dst nodes (= graphs) contiguously across 8 cores (10240 nodes =
    256 graphs per core).
  - Edges are random over the full node set; each layer gathers source rows
    with gpsimd dma_gather (edges sorted by (src-chunk, dst-tile) on host),
    aggregates per 128-dst tile with one-hot matmuls on TensorE (bf16).
  - Self-loops bypass the gather (sequential stream + PE transpose).
  - Layer 2 is PRUNED to host destinations only (the actor head reads just
    the first 13 of every 40 nodes), cutting L2 edges 40/13 ~ 3x.
  - Between layers: AllGather of the dinv-prescaled h1@W2 ("m2s"), stored
    as bf16 padded to 128 cols (256B rows) so gathered rows feed the PE
    scatter matmuls directly with no per-edge cast.
  - Head: host rows are static; dst-side dinv/bias/relu applied on the
    selected slots only; f32 GEMM + softmax.
"""
import sys
import hashlib

sys.path.insert(0, "/opt/trn_rl_repo")

import numpy as np
import ml_dtypes
from contextlib import ExitStack

from concourse import bass, mybir, tile, bass_utils, bacc
from concourse.masks import make_identity

F32 = mybir.dt.float32
BF16 = mybir.dt.bfloat16
I16 = mybir.dt.int16
I32 = mybir.dt.int32

N_CORES = 8
N = 81920
NL = N // N_CORES          # 10240 nodes per core
IN_DIM = 128
H1 = 256
H2 = 64
GRAPH = 40
NH = 13
ACT = 145
TILES = NL // 128          # 80 dst tiles per core (layer 1)
GPC = NL // GRAPH          # 256 graphs per core
NHL = GPC * NH             # 3328 host nodes per core
T2 = NHL // 128            # 26 dst tiles per core (layer 2, hosts only)
CHUNK = 32768
CHUNKS = [(0, 32768), (32768, 32768), (65536, 16384)]
NCH = 3
CALL_G = 16                # groups (of 128 idxs) per dma_gather call
SENT = 300.0               # sentinel dst value for padding slots


# ---------------------------------------------------------------- host prep

def _plan(core, c_of, t_of, idxl, dloc, coefv, n_tiles):
    """Build the per-core slot array + call/matmul-event schedule for one
    edge set (edges described by per-edge core/chunk/tile/local-idx/dst-loc).
    Slot structure is shared across cores (SPMD): per-(chunk,tile) segment
    sizes are the max over cores."""
    counts = np.zeros((N_CORES, NCH, n_tiles), np.int64)
    np.add.at(counts, (core, c_of, t_of), 1)
    Ncm = counts.max(axis=0)                        # [3, n_tiles]
    seg_off = np.zeros((NCH, n_tiles), np.int64)    # global slot offsets
    chunk_base = np.zeros(NCH + 1, np.int64)
    calls = []   # (chunk, slot0_global, n_g, events)
    off = 0
    for c in range(NCH):
        chunk_base[c] = off
        for t in range(n_tiles):
            seg_off[c, t] = off
            off += int(Ncm[c, t])
        off = ((off - chunk_base[c] + 127) // 128 + 0) * 128 + chunk_base[c] \
            if (off - chunk_base[c]) % 128 else off
    chunk_base[NCH] = off
    L = int(off)

    # per-chunk group structure + matmul/drain events
    for c in range(NCH):
        base = int(chunk_base[c])
        S = int(chunk_base[c + 1] - base)
        ngroups = S // 128
        # tile of each group's first slot
        def tile_of(slot):
            # slot is chunk-local
            j = np.searchsorted(seg_off[c] - base, slot, side="right") - 1
            j = max(0, min(n_tiles - 1, int(j)))
            if slot >= int(seg_off[c, j] - base) + int(Ncm[c, j]):
                return -1          # chunk-tail pad region
            return j
        tg = [tile_of(128 * g) for g in range(ngroups)]
        for g in range(ngroups):
            if tg[g] == -1:
                tg[g] = n_tiles - 1  # tail pads: harmless window
        # first/last group of each tile's segment
        g_a = [(int(seg_off[c, t] - base)) // 128 for t in range(n_tiles)]
        g_b = [(int(seg_off[c, t] - base) + int(Ncm[c, t]) - 1) // 128
               for t in range(n_tiles)]
        # build matmul event list in group order
        events_all = []
        for g in range(ngroups):
            t0 = tg[g]
            seg_end = int(seg_off[c, t0] - base) + int(Ncm[c, t0])
            spans = (t0 + 1 < n_tiles) and (128 * (g + 1) > seg_end)
            if spans:
                assert 128 * (g + 1) <= seg_end + int(Ncm[c, t0 + 1]), \
                    "group spans >2 tiles"
            evs = [(g, 0, t0, g == g_a[t0], g == g_b[t0])]
            if spans:
                t1 = t0 + 1
                evs.append((g, 1, t1, g == g_a[t1], g == g_b[t1]))
            events_all.append(evs)
        # slice into calls
        gi = 0
        while gi < ngroups:
            n = min(CALL_G, ngroups - gi)
            evs = []
            for g in range(gi, gi + n):
                for (gg, half, t, st, sp) in events_all[g]:
                    evs.append((gg - gi, half, t, st, sp))
            calls.append((c, base + 128 * gi, n, evs))
            gi += n

    idx_all = np.zeros((N_CORES, L), np.int16)
    dstv_all = np.full((N_CORES, L), SENT, np.float32)
    coef_all = np.zeros((N_CORES, L), np.float32)
    # group tile map per global slot (for relative dstv)
    tg_of_slot = np.full(L, -1, np.int64)
    for c in range(NCH):
        base = int(chunk_base[c])
        S = int(chunk_base[c + 1] - base)
        for g in range(S // 128):
            j = np.searchsorted(seg_off[c] - base, 128 * g, side="right") - 1
            j = max(0, min(n_tiles - 1, int(j)))
            if 128 * g >= int(seg_off[c, j] - base) + int(Ncm[c, j]):
                j = n_tiles - 1
            tg_of_slot[base + 128 * g: base + 128 * (g + 1)] = j
    for r in range(N_CORES):
        m = core == r
        sc, st = c_of[m], t_of[m]
        si, sd = idxl[m], dloc[m]
        scf = coefv[m] if coefv is not None else None
        order = np.lexsort((st, sc))
        sc, st = sc[order], st[order]
        si, sd = si[order], sd[order]
        key = sc * n_tiles + st
        change = np.r_[True, key[1:] != key[:-1]]
        starts = np.flatnonzero(change)
        runid = np.cumsum(change) - 1
        within = np.arange(len(key)) - starts[runid]
        base_run = seg_off[sc[starts], st[starts]]
        pos = base_run[runid] + within
        idx_all[r, pos] = si.astype(np.int16)
        dstv_all[r, pos] = sd + 128.0 * (st - tg_of_slot[pos])
        if scf is not None:
            coef_all[r, pos] = scf[order]

    idx_sb = np.stack([
        np.tile(idx_all[r].reshape(-1, 16).T, (8, 1)) for r in range(N_CORES)
    ])                                               # [8, 128, L/16]
    dstv_sb = np.stack([
        dstv_all[r].reshape(-1, 128).T for r in range(N_CORES)
    ]).astype(ml_dtypes.bfloat16)                    # [8, 128, L/128]
    coef_sb = np.stack([
        coef_all[r].reshape(-1, 128).T for r in range(N_CORES)
    ])                                               # [8, 128, L/128]
    return L, calls, idx_sb, dstv_sb, coef_sb


def _prep(ei):
    src = ei[0].astype(np.int64)
    dst = ei[1].astype(np.int64)
    deg = np.bincount(dst, minlength=N).astype(np.float64) + 1.0
    dinv = (1.0 / np.sqrt(deg)).astype(np.float32)
    coef = (dinv[src] * dinv[dst]).astype(np.float32)

    # ---- layer-1 plan: all edges, dst tiles over all local nodes
    core = dst // NL
    t_of = (dst % NL) // 128
    c_of = src // CHUNK
    idxl = src % CHUNK
    dloc = (dst % 128).astype(np.float32)
    L, calls, idx_sb, dstv_sb, coef_sb = _plan(
        core, c_of, t_of, idxl, dloc, coef, TILES)

    # ---- layer-2 plan: host-dst edges only, dst tiles over host slots
    hmask = (dst % GRAPH) < NH
    src2, dst2 = src[hmask], dst[hmask]
    core2 = dst2 // NL
    hostloc = (dst2 % NL) // GRAPH * NH + dst2 % GRAPH
    t2_of = hostloc // 128
    c2_of = src2 // CHUNK
    idxl2 = src2 % CHUNK
    dloc2 = (hostloc % 128).astype(np.float32)
    L2, calls2, idx2_sb, dstv2_sb, _ = _plan(
        core2, c2_of, t2_of, idxl2, dloc2, None, T2)

    dinv_l = dinv.reshape(N_CORES, NL)
    dinv_tiles = np.ascontiguousarray(
        dinv_l.reshape(N_CORES, TILES, 128).transpose(0, 2, 1))   # [8,128,80]
    dinv2_tiles = (dinv_tiles ** 2).astype(np.float32)

    # per-slot dst dinv for the head: hzT[p, k, g] -> host h=2k+(p>=64),
    # feat=p%64, local node g*40+h
    dinv_hz = np.zeros((N_CORES, 128, 7, GPC), np.float32)
    for k in range(7):
        for half in range(2):
            h = 2 * k + half
            if h >= NH:
                continue
            nodes = np.arange(GPC) * GRAPH + h
            dinv_hz[:, 64 * half:64 * (half + 1), k, :] = \
                dinv_l[:, nodes][:, None, :]

    return dict(L=L, calls=calls, idx_sb=idx_sb, dstv_sb=dstv_sb,
                coef_sb=coef_sb, L2=L2, calls2=calls2, idx2_sb=idx2_sb,
                dstv2_sb=dstv2_sb, dinv_tiles=dinv_tiles,
                dinv2_tiles=dinv2_tiles, dinv_hz=dinv_hz)


# ---------------------------------------------------------------- builder

def _edge_pass(nc, wk, psA, calls, src_dram, elem, idxt, dstvt, coeft,
               iota_bf, agg, selfT, l2):
    """Shared edge-aggregation pass for both layers (256-dst windows).

    l2=False: gather f32 rows, scale by per-edge coef -> bf16 lhsT.
    l2=True: gather bf16 rows (padded to `elem`), first 64 cols are the
    payload and feed the PE directly (no per-edge vector op)."""
    open_ps = {}
    gdt = BF16 if l2 else F32
    for (c, slot0, n_g, events) in calls:
        rows0, nrows = CHUNKS[c]
        gat = wk.tile([128, CALL_G, elem], gdt, tag="gat", bufs=3)
        nc.gpsimd.dma_gather(
            out_ap=gat[:, 0:n_g, :],
            in_ap=src_dram[rows0:rows0 + nrows, :],
            idxs_ap=idxt[:, slot0 // 16: slot0 // 16 + n_g * 8],
            num_idxs=n_g * 128, num_idxs_reg=n_g * 128,
            elem_size=elem, single_packet=False)
        s0 = slot0 // 128
        if l2:
            gatb = gat
        else:
            gatb = wk.tile([128, CALL_G, elem], BF16, tag="gatb", bufs=3)
            nc.vector.tensor_tensor(
                out=gatb[:, 0:n_g, :], in0=gat[:, 0:n_g, :],
                in1=coeft[:, s0:s0 + n_g].unsqueeze(2).to_broadcast(
                    (128, n_g, elem)),
                op=mybir.AluOpType.mult)
        oh = wk.tile([128, CALL_G, 256], BF16, tag="oh", bufs=3)
        nc.vector.tensor_tensor(
            out=oh[:, 0:n_g, :],
            in0=dstvt[:, s0:s0 + n_g].unsqueeze(2).to_broadcast((128, n_g, 256)),
            in1=iota_bf[:].unsqueeze(1).to_broadcast((128, n_g, 256)),
            op=mybir.AluOpType.is_equal)
        M = 64 if l2 else 128
        for (g, half, t, first, last) in events:
            if first:
                open_ps[t] = psA.tile([M, 128], F32, tag="agg",
                                      name=f"aggps_c{c}_t{t}")
            ps = open_ps[t]
            lhsT = gatb[:, g, 0:64] if l2 else gatb[:, g, :]
            nc.tensor.matmul(out=ps[:], lhsT=lhsT,
                             rhs=oh[:, g, 128 * half:128 * (half + 1)],
                             start=first, stop=last)
            if last:
                sl = slice(128 * t, 128 * (t + 1))
                if c == 0:
                    nc.vector.tensor_tensor(out=agg[:, sl], in0=ps[:],
                                            in1=selfT[:, sl],
                                            op=mybir.AluOpType.add)
                else:
                    nc.vector.tensor_tensor(out=agg[:, sl], in0=agg[:, sl],
                                            in1=ps[:], op=mybir.AluOpType.add)
                del open_ps[t]


def _build(L, calls, L2, calls2):
    nc = bacc.Bacc("TRN2", target_bir_lowering=False, debug=False,
                   num_devices=N_CORES)
    d_xfull = nc.dram_tensor("xfull", [N, IN_DIM], F32, kind="ExternalInput")
    d_xloc = nc.dram_tensor("xloc", [NL, IN_DIM], F32, kind="ExternalInput")
    d_idx = nc.dram_tensor("idx", [128, L // 16], I16, kind="ExternalInput")
    d_dstv = nc.dram_tensor("dstv", [128, L // 128], BF16, kind="ExternalInput")
    d_coef = nc.dram_tensor("coef", [128, L // 128], F32, kind="ExternalInput")
    d_idx2 = nc.dram_tensor("idx2", [128, L2 // 16], I16, kind="ExternalInput")
    d_dstv2 = nc.dram_tensor("dstv2", [128, L2 // 128], BF16,
                             kind="ExternalInput")
    d_dinvt = nc.dram_tensor("dinvt", [128, TILES], F32, kind="ExternalInput")
    d_dinv2t = nc.dram_tensor("dinv2t", [128, TILES], F32, kind="ExternalInput")
    d_dinvhz = nc.dram_tensor("dinvhz", [128, 7 * GPC], F32, kind="ExternalInput")
    d_W1 = nc.dram_tensor("W1", [IN_DIM, H1], F32, kind="ExternalInput")
    d_b1 = nc.dram_tensor("b1", [H1, 1], F32, kind="ExternalInput")
    d_W2 = nc.dram_tensor("W2", [H1, H2], F32, kind="ExternalInput")
    d_b2hz = nc.dram_tensor("b2hz", [128, 1], F32, kind="ExternalInput")
    d_Wout = nc.dram_tensor("Wout", [NH * H2, ACT], F32, kind="ExternalInput")
    d_bout = nc.dram_tensor("bout", [1, ACT], F32, kind="ExternalInput")
    d_out = nc.dram_tensor("out", [GPC, ACT], F32, kind="ExternalOutput")

    with tile.TileContext(nc) as tc, ExitStack() as top:
        perm = top.enter_context(tc.tile_pool(name="perm", bufs=1))
        dram = top.enter_context(tc.tile_pool(name="dram", bufs=1, space="DRAM"))

        # ---- persistent tiles
        idxt = perm.tile([128, L // 16], I16)
        nc.sync.dma_start(out=idxt[:], in_=d_idx[:])
        dstvt = perm.tile([128, L // 128], BF16)
        nc.sync.dma_start(out=dstvt[:], in_=d_dstv[:])
        coeft = perm.tile([128, L // 128], F32)
        nc.sync.dma_start(out=coeft[:], in_=d_coef[:])
        idxt2 = perm.tile([128, L2 // 16], I16)
        nc.sync.dma_start(out=idxt2[:], in_=d_idx2[:])
        dstvt2 = perm.tile([128, L2 // 128], BF16)
        nc.sync.dma_start(out=dstvt2[:], in_=d_dstv2[:])
        dinvt = perm.tile([128, TILES], F32)
        nc.sync.dma_start(out=dinvt[:], in_=d_dinvt[:])
        dinv2t = perm.tile([128, TILES], F32)
        nc.sync.dma_start(out=dinv2t[:], in_=d_dinv2t[:])
        W1sb = perm.tile([128, H1], BF16)
        nc.gpsimd.dma_start(out=W1sb[:], in_=d_W1[:])
        b1sb = perm.tile([128, 2], F32)
        nc.sync.dma_start(out=b1sb[:, 0:1], in_=d_b1[0:128, :])
        nc.sync.dma_start(out=b1sb[:, 1:2], in_=d_b1[128:256, :])
        W2sb = perm.tile([128, 2, H2], BF16)
        nc.gpsimd.dma_start(out=W2sb[:, 0, :], in_=d_W2[0:128, :])
        nc.gpsimd.dma_start(out=W2sb[:, 1, :], in_=d_W2[128:256, :])
        b2hz = perm.tile([128, 1], F32)
        nc.sync.dma_start(out=b2hz[:], in_=d_b2hz[:])
        WoutSB = perm.tile([128, 7, ACT], F32)
        for k in range(6):
            nc.sync.dma_start(out=WoutSB[:, k, :],
                              in_=d_Wout[128 * k:128 * (k + 1), :])
        nc.sync.dma_start(out=WoutSB[0:64, 6, :], in_=d_Wout[768:832, :])
        boutrep = perm.tile([128, ACT], F32)
        nc.sync.dma_start(out=boutrep[:], in_=d_bout[:].to_broadcast((128, ACT)))
        dinvhz = perm.tile([128, 7, GPC], F32)
        nc.sync.dma_start(out=dinvhz[:].rearrange("p k g -> p (k g)"),
                          in_=d_dinvhz[:])

        ident = perm.tile([128, 128], F32)
        make_identity(nc, ident[:])
        identb = perm.tile([128, 128], BF16)
        nc.vector.tensor_copy(out=identb[:], in_=ident[:])
        iota_i = perm.tile([128, 256], I32)
        nc.gpsimd.iota(iota_i[:], pattern=[[1, 256]], base=0,
                       channel_multiplier=0)
        iota_bf = perm.tile([128, 256], BF16)
        nc.vector.tensor_copy(out=iota_bf[:], in_=iota_i[:])
        iota_f = perm.tile([128, 128], F32)
        nc.vector.tensor_copy(out=iota_f[:], in_=iota_i[:, 0:128])
        # ident_hi[p, j] = 1 if j == p + 64 (used to shift rows up by 64)
        ioc = perm.tile([128, 1], I32)
        nc.gpsimd.iota(ioc[:], pattern=[[1, 1]], base=64, channel_multiplier=1)
        iocf = perm.tile([128, 1], F32)
        nc.vector.tensor_copy(out=iocf[:], in_=ioc[:])
        ident_hi = perm.tile([128, 128], F32)
        nc.vector.tensor_tensor(out=ident_hi[:],
                                in0=iocf[:].to_broadcast((128, 128)),
                                in1=iota_f[:], op=mybir.AluOpType.is_equal)

        h1T = perm.tile([128, 2, NL], BF16)

        # =========================== Layer 1 ===========================
        with ExitStack() as ph1:
            mid1 = ph1.enter_context(tc.tile_pool(name="mid1", bufs=1))
            wk1 = ph1.enter_context(tc.tile_pool(name="wk1", bufs=2))
            psA = ph1.enter_context(tc.tile_pool(name="psA", bufs=4,
                                                 space="PSUM"))
            psT = ph1.enter_context(tc.tile_pool(name="psT", bufs=2,
                                                 space="PSUM"))

            agg1 = mid1.tile([128, NL], BF16)
            xTs = mid1.tile([128, NL], BF16)
            for t in range(TILES):
                xl = wk1.tile([128, 128], F32, tag="xl")
                nc.sync.dma_start(out=xl[:], in_=d_xloc[128 * t:128 * (t + 1), :])
                xls = wk1.tile([128, 128], F32, tag="xls")
                nc.vector.tensor_scalar_mul(xls[:], xl[:], dinv2t[:, t:t + 1])
                pt = psT.tile([128, 128], F32, tag="tr")
                nc.tensor.transpose(out=pt[:], in_=xls[:], identity=ident[:])
                nc.scalar.activation(out=xTs[:, 128 * t:128 * (t + 1)],
                                     in_=pt[:],
                                     func=mybir.ActivationFunctionType.Copy)

            _edge_pass(nc, wk1, psA, calls, d_xfull, IN_DIM, idxt, dstvt,
                       coeft, iota_bf, agg1, xTs, l2=False)

            with tc.tile_pool(name="psG1", bufs=2, space="PSUM") as psG:
                for m in range(2):
                    for nb in range(NL // 512):
                        pg = psG.tile([128, 512], F32, tag="g1")
                        nc.tensor.matmul(
                            out=pg[:], lhsT=W1sb[:, 128 * m:128 * (m + 1)],
                            rhs=agg1[:, 512 * nb:512 * (nb + 1)],
                            start=True, stop=True)
                        nc.scalar.activation(
                            out=h1T[:, m, 512 * nb:512 * (nb + 1)], in_=pg[:],
                            func=mybir.ActivationFunctionType.Relu,
                            bias=b1sb[:, m:m + 1], scale=1.0)

        # ================== GEMM2 + m2s + AllGather ====================
        # m2s rows are bf16 padded to 128 cols (256B) so layer-2 gathers
        # feed the PE directly.
        m2sl = dram.tile([NL, 128], BF16)
        m2sf = dram.tile([N, 128], BF16, addr_space="Shared")
        with ExitStack() as ph2:
            midA = ph2.enter_context(tc.tile_pool(name="midA", bufs=1))
            m2sTs = midA.tile([64, NL], BF16)
            selfTh = midA.tile([64, NHL], BF16)
            agg2 = midA.tile([64, NHL], F32)
            with ExitStack() as ph2a:
                mid2 = ph2a.enter_context(tc.tile_pool(name="mid2", bufs=1))
                psG2 = ph2a.enter_context(tc.tile_pool(name="psG2", bufs=2,
                                                       space="PSUM"))
                psT2 = ph2a.enter_context(tc.tile_pool(name="psT2", bufs=2,
                                                       space="PSUM"))
                stage = mid2.tile([128, TILES, 128], BF16)
                nc.vector.memset(stage[:, :, 64:128], 0.0)
                for t in range(TILES):
                    pg = psG2.tile([128, H2], F32, tag="g2")
                    for m in range(2):
                        nc.tensor.matmul(
                            out=pg[:], lhsT=h1T[:, m, 128 * t:128 * (t + 1)],
                            rhs=W2sb[:, m, :], start=(m == 0), stop=(m == 1))
                    nc.vector.tensor_scalar_mul(stage[:, t, 0:64], pg[:],
                                                dinvt[:, t:t + 1])
                for t in range(TILES):
                    pt = psT2.tile([64, 128], BF16, tag="tr2")
                    nc.tensor.transpose(out=pt[:], in_=stage[:, t, 0:64],
                                        identity=identb[:])
                    nc.scalar.activation(
                        out=m2sTs[:, 128 * t:128 * (t + 1)], in_=pt[:],
                        func=mybir.ActivationFunctionType.Copy)
                nc.sync.dma_start(
                    out=m2sl[:].rearrange("(t p) f -> p t f", p=128),
                    in_=stage[:])
            # self-loop payload at host slots: hostloc g*13+h <- node g*40+h
            sTh = selfTh[:].rearrange("p (g q) -> p q g", q=NH)
            mTs = m2sTs[:].rearrange("p (g q) -> p q g", q=GRAPH)
            for h in range(NH):
                nc.vector.tensor_copy(out=sTh[:, h, :], in_=mTs[:, h, :])
            nc.gpsimd.collective_compute(
                "AllGather", mybir.AluOpType.bypass,
                replica_groups=[list(range(N_CORES))],
                ins=[m2sl[:].opt()], outs=[m2sf[:].opt()])

            # ========================= Layer 2 =========================
            with ExitStack() as ph3:
                wk2 = ph3.enter_context(tc.tile_pool(name="wk2", bufs=2))
                psA2 = ph3.enter_context(tc.tile_pool(name="psA2", bufs=4,
                                                      space="PSUM"))
                _edge_pass(nc, wk2, psA2, calls2, m2sf, 128, idxt2, dstvt2,
                           None, iota_bf, agg2, selfTh, l2=True)

            # ===================== actor head ==========================
            with ExitStack() as ph4:
                mid4 = ph4.enter_context(tc.tile_pool(name="mid4", bufs=1))
                wk4 = ph4.enter_context(tc.tile_pool(name="wk4", bufs=2))
                psF = ph4.enter_context(tc.tile_pool(name="psF", bufs=2,
                                                     space="PSUM"))
                hzT = mid4.tile([128, 7, GPC], F32)
                h2r = agg2[:].rearrange("p (g q) -> p q g", q=NH)
                for k in range(7):
                    pk = psF.tile([128, GPC], F32, tag="hz", name=f"hzps{k}")
                    nc.tensor.matmul(out=pk[:], lhsT=ident[0:64, :],
                                     rhs=h2r[:, 2 * k, :],
                                     start=True, stop=(k == 6))
                    if k < 6:
                        nc.tensor.matmul(out=pk[:], lhsT=ident_hi[0:64, :],
                                         rhs=h2r[:, 2 * k + 1, :],
                                         start=False, stop=True)
                    nc.vector.tensor_tensor(out=hzT[:, k, :], in0=pk[:],
                                            in1=dinvhz[:, k, :],
                                            op=mybir.AluOpType.mult)
                nc.scalar.activation(out=hzT[:].rearrange("p k g -> p (k g)"),
                                     in_=hzT[:].rearrange("p k g -> p (k g)"),
                                     func=mybir.ActivationFunctionType.Relu,
                                     bias=b2hz[:, 0:1], scale=1.0)
                for m in range(GPC // 128):
                    pf = psF.tile([128, ACT], F32, tag="fin")
                    for k in range(6):
                        nc.tensor.matmul(
                            out=pf[:], lhsT=hzT[:, k, 128 * m:128 * (m + 1)],
                            rhs=WoutSB[:, k, :], start=(k == 0), stop=False)
                    nc.tensor.matmul(
                        out=pf[:], lhsT=hzT[0:64, 6, 128 * m:128 * (m + 1)],
                        rhs=WoutSB[0:64, 6, :], start=False, stop=True)
                    nc.vector.tensor_tensor(out=pf[:], in0=pf[:],
                                            in1=boutrep[:],
                                            op=mybir.AluOpType.add)
                    mx = wk4.tile([128, 1], F32, tag="mx")
                    nc.vector.tensor_reduce(out=mx[:], in_=pf[:],
                                            axis=mybir.AxisListType.X,
                                            op=mybir.AluOpType.max)
                    nmx = wk4.tile([128, 1], F32, tag="nmx")
                    nc.vector.tensor_scalar_mul(nmx[:], mx[:], -1.0)
                    esb = wk4.tile([128, ACT], F32, tag="esb")
                    nc.scalar.activation(out=esb[:], in_=pf[:],
                                         func=mybir.ActivationFunctionType.Exp,
                                         bias=nmx[:, 0:1], scale=1.0)
                    ssum = wk4.tile([128, 1], F32, tag="ssum")
                    nc.vector.tensor_reduce(out=ssum[:], in_=esb[:],
                                            axis=mybir.AxisListType.X,
                                            op=mybir.AluOpType.add)
                    rcp = wk4.tile([128, 1], F32, tag="rcp")
                    nc.vector.reciprocal(out=rcp[:], in_=ssum[:])
                    osb = wk4.tile([128, ACT], F32, tag="osb")
                    nc.vector.tensor_scalar_mul(osb[:], esb[:], rcp[:, 0:1])
                    nc.sync.dma_start(out=d_out[128 * m:128 * (m + 1), :],
                                      in_=osb[:])

    nc.compile()
    return nc


# ---------------------------------------------------------------- entry

_CACHE = {}


def _get(x, ei):
    key = hashlib.sha1(ei.tobytes()).hexdigest()
    if key not in _CACHE:
        meta = _prep(ei)
        nc = _build(meta["L"], meta["calls"], meta["L2"], meta["calls2"])
        _CACHE[key] = (meta, nc)
    return _CACHE[key]


def _in_maps(meta, x, W1, b1, W2, b2, Wout, bout):
    b2t = np.tile(np.asarray(b2, np.float32).reshape(H2), 2).reshape(128, 1)
    maps = []
    for r in range(N_CORES):
        maps.append({
            "xfull": x,
            "xloc": np.ascontiguousarray(x[r * NL:(r + 1) * NL, :]),
            "idx": np.ascontiguousarray(meta["idx_sb"][r]),
            "dstv": np.ascontiguousarray(meta["dstv_sb"][r]),
            "coef": np.ascontiguousarray(meta["coef_sb"][r]),
            "idx2": np.ascontiguousarray(meta["idx2_sb"][r]),
            "dstv2": np.ascontiguousarray(meta["dstv2_sb"][r]),
            "dinvt": np.ascontiguousarray(meta["dinv_tiles"][r]),
            "dinv2t": np.ascontiguousarray(meta["dinv2_tiles"][r]),
            "dinvhz": np.ascontiguousarray(
                meta["dinv_hz"][r].reshape(128, 7 * GPC)),
            "W1": np.ascontiguousarray(W1, np.float32),
            "b1": np.ascontiguousarray(b1, np.float32).reshape(H1, 1),
            "W2": np.ascontiguousarray(W2, np.float32),
            "b2hz": b2t,
            "Wout": np.ascontiguousarray(Wout, np.float32),
            "bout": np.ascontiguousarray(bout, np.float32).reshape(1, ACT),
        })
    return maps


def kernel(x, ei, W1, b1, W2, b2, Wout, bout, _trace=False):
    x = np.ascontiguousarray(x, np.float32)
    ei = np.ascontiguousarray(ei, np.int32)
    meta, nc = _get(x, ei)
    maps = _in_maps(meta, x, W1, b1, W2, b2, Wout, bout)
    res = bass_utils.run_bass_kernel_spmd(
        nc, maps, core_ids=list(range(N_CORES)), trace=_trace)
    out = np.concatenate([res.results[r]["out"] for r in range(N_CORES)],
                         axis=0).astype(np.float32)
    if _trace:
        return out, res.exec_time_ns
    return out


def install_profile_hook():
    import types
    sys.path.insert(0, "/root/.axon_site")
    import trn_agent_boot.trn_boot as _tb
    import antenv
    if "antenv.axon_hooks" not in sys.modules:
        _mod = types.ModuleType("antenv.axon_hooks")
        _h = [None]
        _mod.set_axon_ntff_profile_hook = lambda h: _h.__setitem__(0, h)
        _mod.get_axon_ntff_profile_hook = lambda: _h[0]
        sys.modules["antenv.axon_hooks"] = _mod
        antenv.axon_hooks = _mod
        _mod.set_axon_ntff_profile_hook(
            _tb._ntff_profile_via_ctypes("/opt/axon/libaxon_pjrt.so"))


# revision 5
# speedup vs baseline: 1.5202x; 1.0942x over previous
"""Trainium2 Bass kernel for nn_ActorNetwork (2-layer GCN + actor head).

Self-contained: hardcodes all shapes/sharding (8 NeuronCores).

Strategy:
  - Shard dst nodes (= graphs) contiguously across 8 cores (10240 nodes =
    256 graphs per core).
  - Edges are random over the full node set; each layer gathers source rows
    with gpsimd dma_gather (edges sorted by (src-chunk, dst-tile) on host),
    aggregates per 128-dst tile with one-hot matmuls on TensorE (bf16).
  - Self-loops bypass the gather (sequential stream + PE transpose).
  - Layer 2 is PRUNED to host destinations only (the actor head reads just
    the first 13 of every 40 nodes), cutting L2 edges 40/13 ~ 3x.
  - Between layers: AllGather of the dinv-prescaled h1@W2 ("m2s"), stored
    as bf16 padded to 128 cols (256B rows) so gathered rows feed the PE
    scatter matmuls directly with no per-edge cast.
  - Head: host rows are static; dst-side dinv/bias/relu applied on the
    selected slots only; f32 GEMM + softmax.
"""
import sys
import hashlib

sys.path.insert(0, "/opt/trn_rl_repo")

import numpy as np
import ml_dtypes
from contextlib import ExitStack

from concourse import bass, mybir, tile, bass_utils, bacc
from concourse.masks import make_identity

F32 = mybir.dt.float32
BF16 = mybir.dt.bfloat16
I16 = mybir.dt.int16
I32 = mybir.dt.int32

N_CORES = 8
N = 81920
NL = N // N_CORES          # 10240 nodes per core
IN_DIM = 128
H1 = 256
H2 = 64
GRAPH = 40
NH = 13
ACT = 145
TILES = NL // 128          # 80 dst tiles per core (layer 1)
GPC = NL // GRAPH          # 256 graphs per core
NHL = GPC * NH             # 3328 host nodes per core
T2 = NHL // 128            # 26 dst tiles per core (layer 2, hosts only)
CHUNK = 32768
CHUNKS = [(0, 32768), (32768, 32768), (65536, 16384)]
NCH = 3
CALL_G = 16                # groups (of 128 idxs) per dma_gather call
SENT = 300.0               # sentinel dst value for padding slots


# ---------------------------------------------------------------- host prep

def _plan(core, c_of, t_of, idxl, dloc, coefv, n_tiles):
    """Build the per-core slot array + call/matmul-event schedule for one
    edge set (edges described by per-edge core/chunk/tile/local-idx/dst-loc).
    Slot structure is shared across cores (SPMD): per-(chunk,tile) segment
    sizes are the max over cores."""
    counts = np.zeros((N_CORES, NCH, n_tiles), np.int64)
    np.add.at(counts, (core, c_of, t_of), 1)
    Ncm = counts.max(axis=0)                        # [3, n_tiles]
    seg_off = np.zeros((NCH, n_tiles), np.int64)    # global slot offsets
    chunk_base = np.zeros(NCH + 1, np.int64)
    calls = []   # (chunk, slot0_global, n_g, events)
    off = 0
    for c in range(NCH):
        chunk_base[c] = off
        for t in range(n_tiles):
            seg_off[c, t] = off
            off += int(Ncm[c, t])
        off = ((off - chunk_base[c] + 127) // 128 + 0) * 128 + chunk_base[c] \
            if (off - chunk_base[c]) % 128 else off
    chunk_base[NCH] = off
    L = int(off)

    # per-chunk group structure + matmul/drain events
    for c in range(NCH):
        base = int(chunk_base[c])
        S = int(chunk_base[c + 1] - base)
        ngroups = S // 128
        # tile of each group's first slot
        def tile_of(slot):
            # slot is chunk-local
            j = np.searchsorted(seg_off[c] - base, slot, side="right") - 1
            j = max(0, min(n_tiles - 1, int(j)))
            if slot >= int(seg_off[c, j] - base) + int(Ncm[c, j]):
                return -1          # chunk-tail pad region
            return j
        tg = [tile_of(128 * g) for g in range(ngroups)]
        for g in range(ngroups):
            if tg[g] == -1:
                tg[g] = n_tiles - 1  # tail pads: harmless window
        # first/last group of each tile's segment
        g_a = [(int(seg_off[c, t] - base)) // 128 for t in range(n_tiles)]
        g_b = [(int(seg_off[c, t] - base) + int(Ncm[c, t]) - 1) // 128
               for t in range(n_tiles)]
        # build matmul event list in group order
        events_all = []
        for g in range(ngroups):
            t0 = tg[g]
            seg_end = int(seg_off[c, t0] - base) + int(Ncm[c, t0])
            spans = (t0 + 1 < n_tiles) and (128 * (g + 1) > seg_end)
            if spans:
                assert 128 * (g + 1) <= seg_end + int(Ncm[c, t0 + 1]), \
                    "group spans >2 tiles"
            evs = [(g, 0, t0, g == g_a[t0], g == g_b[t0])]
            if spans:
                t1 = t0 + 1
                evs.append((g, 1, t1, g == g_a[t1], g == g_b[t1]))
            events_all.append(evs)
        # slice into calls
        gi = 0
        while gi < ngroups:
            n = min(CALL_G, ngroups - gi)
            evs = []
            for g in range(gi, gi + n):
                for (gg, half, t, st, sp) in events_all[g]:
                    evs.append((gg - gi, half, t, st, sp))
            calls.append((c, base + 128 * gi, n, evs))
            gi += n

    idx_all = np.zeros((N_CORES, L), np.int16)
    dstv_all = np.full((N_CORES, L), SENT, np.float32)
    coef_all = np.zeros((N_CORES, L), np.float32)
    # group tile map per global slot (for relative dstv)
    tg_of_slot = np.full(L, -1, np.int64)
    for c in range(NCH):
        base = int(chunk_base[c])
        S = int(chunk_base[c + 1] - base)
        for g in range(S // 128):
            j = np.searchsorted(seg_off[c] - base, 128 * g, side="right") - 1
            j = max(0, min(n_tiles - 1, int(j)))
            if 128 * g >= int(seg_off[c, j] - base) + int(Ncm[c, j]):
                j = n_tiles - 1
            tg_of_slot[base + 128 * g: base + 128 * (g + 1)] = j
    for r in range(N_CORES):
        m = core == r
        sc, st = c_of[m], t_of[m]
        si, sd = idxl[m], dloc[m]
        scf = coefv[m] if coefv is not None else None
        order = np.lexsort((st, sc))
        sc, st = sc[order], st[order]
        si, sd = si[order], sd[order]
        key = sc * n_tiles + st
        change = np.r_[True, key[1:] != key[:-1]]
        starts = np.flatnonzero(change)
        runid = np.cumsum(change) - 1
        within = np.arange(len(key)) - starts[runid]
        base_run = seg_off[sc[starts], st[starts]]
        pos = base_run[runid] + within
        idx_all[r, pos] = si.astype(np.int16)
        dstv_all[r, pos] = sd + 128.0 * (st - tg_of_slot[pos])
        if scf is not None:
            coef_all[r, pos] = scf[order]

    idx_sb = np.stack([
        np.tile(idx_all[r].reshape(-1, 16).T, (8, 1)) for r in range(N_CORES)
    ])                                               # [8, 128, L/16]
    dstv_sb = np.stack([
        dstv_all[r].reshape(-1, 128).T for r in range(N_CORES)
    ]).astype(ml_dtypes.bfloat16)                    # [8, 128, L/128]
    coef_sb = np.stack([
        coef_all[r].reshape(-1, 128).T for r in range(N_CORES)
    ])                                               # [8, 128, L/128]
    return L, calls, idx_sb, dstv_sb, coef_sb


def _prep(ei):
    src = ei[0].astype(np.int64)
    dst = ei[1].astype(np.int64)
    deg = np.bincount(dst, minlength=N).astype(np.float64) + 1.0
    dinv = (1.0 / np.sqrt(deg)).astype(np.float32)
    coef = (dinv[src] * dinv[dst]).astype(np.float32)

    # ---- layer-1 plan: all edges, dst tiles over all local nodes
    core = dst // NL
    t_of = (dst % NL) // 128
    c_of = src // CHUNK
    idxl = src % CHUNK
    dloc = (dst % 128).astype(np.float32)
    L, calls, idx_sb, dstv_sb, coef_sb = _plan(
        core, c_of, t_of, idxl, dloc, coef, TILES)

    # ---- layer-2 plan: host-dst edges only, dst tiles over host slots
    hmask = (dst % GRAPH) < NH
    src2, dst2 = src[hmask], dst[hmask]
    core2 = dst2 // NL
    hostloc = (dst2 % NL) // GRAPH * NH + dst2 % GRAPH
    t2_of = hostloc // 128
    c2_of = src2 // CHUNK
    idxl2 = src2 % CHUNK
    dloc2 = (hostloc % 128).astype(np.float32)
    L2, calls2, idx2_sb, dstv2_sb, _ = _plan(
        core2, c2_of, t2_of, idxl2, dloc2, None, T2)

    dinv_l = dinv.reshape(N_CORES, NL)
    dinv_tiles = np.ascontiguousarray(
        dinv_l.reshape(N_CORES, TILES, 128).transpose(0, 2, 1))   # [8,128,80]
    dinv2_tiles = (dinv_tiles ** 2).astype(np.float32)

    # per-slot dst dinv for the head: hzT[p, k, g] -> host h=2k+(p>=64),
    # feat=p%64, local node g*40+h
    dinv_hz = np.zeros((N_CORES, 128, 7, GPC), np.float32)
    for k in range(7):
        for half in range(2):
            h = 2 * k + half
            if h >= NH:
                continue
            nodes = np.arange(GPC) * GRAPH + h
            dinv_hz[:, 64 * half:64 * (half + 1), k, :] = \
                dinv_l[:, nodes][:, None, :]

    return dict(L=L, calls=calls, idx_sb=idx_sb, dstv_sb=dstv_sb,
                coef_sb=coef_sb, L2=L2, calls2=calls2, idx2_sb=idx2_sb,
                dstv2_sb=dstv2_sb, dinv_tiles=dinv_tiles,
                dinv2_tiles=dinv2_tiles, dinv_hz=dinv_hz)


# ---------------------------------------------------------------- builder

def _edge_pass(nc, wk, psA, calls, src_dram, elem, idxt, dstvt, coeft,
               iota_bf, agg, selfT, l2, self_lhs=None):
    """Shared edge-aggregation pass for both layers (256-dst windows).

    l2=False: gather f32 rows, scale by per-edge coef -> bf16 lhsT.
    l2=True: gather bf16 rows (padded to `elem`), first 64 cols are the
    payload and feed the PE directly (no per-edge vector op).
    self_lhs=(xb, dg): fold the self-loop term into each tile's psum as an
    extra matmul (xb[:,t,:]^T @ diag) when the tile opens in chunk 0;
    otherwise the self term is added from `selfT` at chunk-0 close."""
    open_ps = {}
    gdt = BF16 if l2 else F32
    for (c, slot0, n_g, events) in calls:
        rows0, nrows = CHUNKS[c]
        gat = wk.tile([128, CALL_G, elem], gdt, tag="gat", bufs=3)
        nc.gpsimd.dma_gather(
            out_ap=gat[:, 0:n_g, :],
            in_ap=src_dram[rows0:rows0 + nrows, :],
            idxs_ap=idxt[:, slot0 // 16: slot0 // 16 + n_g * 8],
            num_idxs=n_g * 128, num_idxs_reg=n_g * 128,
            elem_size=elem, single_packet=False)
        s0 = slot0 // 128
        if l2:
            gatb = gat
        else:
            gatb = wk.tile([128, CALL_G, elem], BF16, tag="gatb", bufs=3)
            nc.vector.tensor_tensor(
                out=gatb[:, 0:n_g, :], in0=gat[:, 0:n_g, :],
                in1=coeft[:, s0:s0 + n_g].unsqueeze(2).to_broadcast(
                    (128, n_g, elem)),
                op=mybir.AluOpType.mult)
        oh = wk.tile([128, CALL_G, 256], BF16, tag="oh", bufs=3)
        nc.vector.tensor_tensor(
            out=oh[:, 0:n_g, :],
            in0=dstvt[:, s0:s0 + n_g].unsqueeze(2).to_broadcast((128, n_g, 256)),
            in1=iota_bf[:].unsqueeze(1).to_broadcast((128, n_g, 256)),
            op=mybir.AluOpType.is_equal)
        M = 64 if l2 else 128
        for (g, half, t, first, last) in events:
            if first:
                open_ps[t] = psA.tile([M, 128], F32, tag="agg",
                                      name=f"aggps_c{c}_t{t}")
                start = True
                if c == 0 and self_lhs is not None:
                    xb, dg = self_lhs
                    nc.tensor.matmul(out=open_ps[t][:], lhsT=xb[:, t, :],
                                     rhs=dg[:, t, :], start=True, stop=False)
                    start = False
            else:
                start = False
            ps = open_ps[t]
            lhsT = gatb[:, g, 0:64] if l2 else gatb[:, g, :]
            nc.tensor.matmul(out=ps[:], lhsT=lhsT,
                             rhs=oh[:, g, 128 * half:128 * (half + 1)],
                             start=start, stop=last)
            if last:
                sl = slice(128 * t, 128 * (t + 1))
                if c == 0:
                    if self_lhs is not None:
                        nc.vector.tensor_copy(out=agg[:, sl], in_=ps[:])
                    else:
                        nc.vector.tensor_tensor(out=agg[:, sl], in0=ps[:],
                                                in1=selfT[:, sl],
                                                op=mybir.AluOpType.add)
                else:
                    nc.vector.tensor_tensor(out=agg[:, sl], in0=agg[:, sl],
                                            in1=ps[:], op=mybir.AluOpType.add)
                del open_ps[t]


def _build(L, calls, L2, calls2):
    nc = bacc.Bacc("TRN2", target_bir_lowering=False, debug=False,
                   num_devices=N_CORES)
    d_xfull = nc.dram_tensor("xfull", [N, IN_DIM], F32, kind="ExternalInput")
    d_xloc = nc.dram_tensor("xloc", [NL, IN_DIM], F32, kind="ExternalInput")
    d_idx = nc.dram_tensor("idx", [128, L // 16], I16, kind="ExternalInput")
    d_dstv = nc.dram_tensor("dstv", [128, L // 128], BF16, kind="ExternalInput")
    d_coef = nc.dram_tensor("coef", [128, L // 128], F32, kind="ExternalInput")
    d_idx2 = nc.dram_tensor("idx2", [128, L2 // 16], I16, kind="ExternalInput")
    d_dstv2 = nc.dram_tensor("dstv2", [128, L2 // 128], BF16,
                             kind="ExternalInput")
    d_dinvt = nc.dram_tensor("dinvt", [128, TILES], F32, kind="ExternalInput")
    d_dinv2t = nc.dram_tensor("dinv2t", [128, TILES], F32, kind="ExternalInput")
    d_dinvhz = nc.dram_tensor("dinvhz", [128, 7 * GPC], F32, kind="ExternalInput")
    d_W1 = nc.dram_tensor("W1", [IN_DIM, H1], F32, kind="ExternalInput")
    d_b1 = nc.dram_tensor("b1", [H1, 1], F32, kind="ExternalInput")
    d_W2 = nc.dram_tensor("W2", [H1, H2], F32, kind="ExternalInput")
    d_b2hz = nc.dram_tensor("b2hz", [128, 1], F32, kind="ExternalInput")
    d_Wout = nc.dram_tensor("Wout", [NH * H2, ACT], F32, kind="ExternalInput")
    d_bout = nc.dram_tensor("bout", [1, ACT], F32, kind="ExternalInput")
    d_out = nc.dram_tensor("out", [GPC, ACT], F32, kind="ExternalOutput")

    with tile.TileContext(nc) as tc, ExitStack() as top:
        perm = top.enter_context(tc.tile_pool(name="perm", bufs=1))
        dram = top.enter_context(tc.tile_pool(name="dram", bufs=1, space="DRAM"))

        # ---- persistent tiles
        idxt = perm.tile([128, L // 16], I16)
        nc.sync.dma_start(out=idxt[:], in_=d_idx[:])
        dstvt = perm.tile([128, L // 128], BF16)
        nc.sync.dma_start(out=dstvt[:], in_=d_dstv[:])
        coeft = perm.tile([128, L // 128], F32)
        nc.sync.dma_start(out=coeft[:], in_=d_coef[:])
        idxt2 = perm.tile([128, L2 // 16], I16)
        nc.sync.dma_start(out=idxt2[:], in_=d_idx2[:])
        dstvt2 = perm.tile([128, L2 // 128], BF16)
        nc.sync.dma_start(out=dstvt2[:], in_=d_dstv2[:])
        dinvt = perm.tile([128, TILES], F32)
        nc.sync.dma_start(out=dinvt[:], in_=d_dinvt[:])
        dinv2t = perm.tile([128, TILES], F32)
        nc.sync.dma_start(out=dinv2t[:], in_=d_dinv2t[:])
        W1sb = perm.tile([128, H1], BF16)
        nc.gpsimd.dma_start(out=W1sb[:], in_=d_W1[:])
        b1sb = perm.tile([128, 2], F32)
        nc.sync.dma_start(out=b1sb[:, 0:1], in_=d_b1[0:128, :])
        nc.sync.dma_start(out=b1sb[:, 1:2], in_=d_b1[128:256, :])
        W2sb = perm.tile([128, 2, H2], BF16)
        nc.gpsimd.dma_start(out=W2sb[:, 0, :], in_=d_W2[0:128, :])
        nc.gpsimd.dma_start(out=W2sb[:, 1, :], in_=d_W2[128:256, :])
        b2hz = perm.tile([128, 1], F32)
        nc.sync.dma_start(out=b2hz[:], in_=d_b2hz[:])
        WoutSB = perm.tile([128, 7, ACT], F32)
        for k in range(6):
            nc.sync.dma_start(out=WoutSB[:, k, :],
                              in_=d_Wout[128 * k:128 * (k + 1), :])
        nc.sync.dma_start(out=WoutSB[0:64, 6, :], in_=d_Wout[768:832, :])
        boutrep = perm.tile([128, ACT], F32)
        nc.sync.dma_start(out=boutrep[:], in_=d_bout[:].to_broadcast((128, ACT)))
        dinvhz = perm.tile([128, 7, GPC], F32)
        nc.sync.dma_start(out=dinvhz[:].rearrange("p k g -> p (k g)"),
                          in_=d_dinvhz[:])

        ident = perm.tile([128, 128], F32)
        make_identity(nc, ident[:])
        identb = perm.tile([128, 128], BF16)
        nc.vector.tensor_copy(out=identb[:], in_=ident[:])
        iota_i = perm.tile([128, 256], I32)
        nc.gpsimd.iota(iota_i[:], pattern=[[1, 256]], base=0,
                       channel_multiplier=0)
        iota_bf = perm.tile([128, 256], BF16)
        nc.vector.tensor_copy(out=iota_bf[:], in_=iota_i[:])
        iota_f = perm.tile([128, 128], F32)
        nc.vector.tensor_copy(out=iota_f[:], in_=iota_i[:, 0:128])
        # ident_hi[p, j] = 1 if j == p + 64 (used to shift rows up by 64)
        ioc = perm.tile([128, 1], I32)
        nc.gpsimd.iota(ioc[:], pattern=[[1, 1]], base=64, channel_multiplier=1)
        iocf = perm.tile([128, 1], F32)
        nc.vector.tensor_copy(out=iocf[:], in_=ioc[:])
        ident_hi = perm.tile([128, 128], F32)
        nc.vector.tensor_tensor(out=ident_hi[:],
                                in0=iocf[:].to_broadcast((128, 128)),
                                in1=iota_f[:], op=mybir.AluOpType.is_equal)

        h1T = perm.tile([128, 2, NL], BF16)

        # =========================== Layer 1 ===========================
        with ExitStack() as ph1:
            mid1 = ph1.enter_context(tc.tile_pool(name="mid1", bufs=1))
            wk1 = ph1.enter_context(tc.tile_pool(name="wk1", bufs=2))
            psA = ph1.enter_context(tc.tile_pool(name="psA", bufs=4,
                                                 space="PSUM"))

            agg1 = mid1.tile([128, NL], BF16)
            # local x as bf16 [p, tile, feat] via one casting DMA; self-loop
            # contribution becomes a per-tile transpose-matmul against a
            # dinv^2-scaled diagonal.
            xbts = mid1.tile([128, TILES, 128], BF16)
            nc.gpsimd.dma_start(
                out=xbts[:], in_=d_xloc[:].rearrange("(t p) f -> p t f", p=128))
            diags = mid1.tile([128, TILES, 128], BF16)
            nc.vector.tensor_tensor(
                out=diags[:],
                in0=identb[:].unsqueeze(1).to_broadcast((128, TILES, 128)),
                in1=dinv2t[:].unsqueeze(2).to_broadcast((128, TILES, 128)),
                op=mybir.AluOpType.mult)

            _edge_pass(nc, wk1, psA, calls, d_xfull, IN_DIM, idxt, dstvt,
                       coeft, iota_bf, agg1, None, l2=False,
                       self_lhs=(xbts, diags))

            with tc.tile_pool(name="psG1", bufs=2, space="PSUM") as psG:
                for m in range(2):
                    for nb in range(NL // 512):
                        pg = psG.tile([128, 512], F32, tag="g1")
                        nc.tensor.matmul(
                            out=pg[:], lhsT=W1sb[:, 128 * m:128 * (m + 1)],
                            rhs=agg1[:, 512 * nb:512 * (nb + 1)],
                            start=True, stop=True)
                        nc.scalar.activation(
                            out=h1T[:, m, 512 * nb:512 * (nb + 1)], in_=pg[:],
                            func=mybir.ActivationFunctionType.Relu,
                            bias=b1sb[:, m:m + 1], scale=1.0)

        # ================== GEMM2 + m2s + AllGather ====================
        # m2s rows are bf16 padded to 128 cols (256B) so layer-2 gathers
        # feed the PE directly.
        m2sl = dram.tile([NL, 128], BF16)
        m2sf = dram.tile([N, 128], BF16, addr_space="Shared")
        with ExitStack() as ph2:
            midA = ph2.enter_context(tc.tile_pool(name="midA", bufs=1))
            m2sTs = midA.tile([64, NL], BF16)
            selfTh = midA.tile([64, NHL], BF16)
            agg2 = midA.tile([64, NHL], F32)
            with ExitStack() as ph2a:
                mid2 = ph2a.enter_context(tc.tile_pool(name="mid2", bufs=1))
                psG2 = ph2a.enter_context(tc.tile_pool(name="psG2", bufs=2,
                                                       space="PSUM"))
                psT2 = ph2a.enter_context(tc.tile_pool(name="psT2", bufs=2,
                                                       space="PSUM"))
                stage = mid2.tile([128, TILES, 128], BF16)
                nc.vector.memset(stage[:, :, 64:128], 0.0)
                for t in range(TILES):
                    pg = psG2.tile([128, H2], F32, tag="g2")
                    for m in range(2):
                        nc.tensor.matmul(
                            out=pg[:], lhsT=h1T[:, m, 128 * t:128 * (t + 1)],
                            rhs=W2sb[:, m, :], start=(m == 0), stop=(m == 1))
                    nc.vector.tensor_scalar_mul(stage[:, t, 0:64], pg[:],
                                                dinvt[:, t:t + 1])
                for t in range(TILES):
                    pt = psT2.tile([64, 128], BF16, tag="tr2")
                    nc.tensor.transpose(out=pt[:], in_=stage[:, t, 0:64],
                                        identity=identb[:])
                    nc.scalar.activation(
                        out=m2sTs[:, 128 * t:128 * (t + 1)], in_=pt[:],
                        func=mybir.ActivationFunctionType.Copy)
                nc.sync.dma_start(
                    out=m2sl[:].rearrange("(t p) f -> p t f", p=128),
                    in_=stage[:])
            # self-loop payload at host slots: hostloc g*13+h <- node g*40+h
            sTh = selfTh[:].rearrange("p (g q) -> p q g", q=NH)
            mTs = m2sTs[:].rearrange("p (g q) -> p q g", q=GRAPH)
            for h in range(NH):
                nc.vector.tensor_copy(out=sTh[:, h, :], in_=mTs[:, h, :])
            nc.gpsimd.collective_compute(
                "AllGather", mybir.AluOpType.bypass,
                replica_groups=[list(range(N_CORES))],
                ins=[m2sl[:].opt()], outs=[m2sf[:].opt()])

            # ========================= Layer 2 =========================
            with ExitStack() as ph3:
                wk2 = ph3.enter_context(tc.tile_pool(name="wk2", bufs=2))
                psA2 = ph3.enter_context(tc.tile_pool(name="psA2", bufs=4,
                                                      space="PSUM"))
                _edge_pass(nc, wk2, psA2, calls2, m2sf, 128, idxt2, dstvt2,
                           None, iota_bf, agg2, selfTh, l2=True)

            # ===================== actor head ==========================
            with ExitStack() as ph4:
                mid4 = ph4.enter_context(tc.tile_pool(name="mid4", bufs=1))
                wk4 = ph4.enter_context(tc.tile_pool(name="wk4", bufs=2))
                psF = ph4.enter_context(tc.tile_pool(name="psF", bufs=2,
                                                     space="PSUM"))
                hzT = mid4.tile([128, 7, GPC], F32)
                h2r = agg2[:].rearrange("p (g q) -> p q g", q=NH)
                for k in range(7):
                    pk = psF.tile([128, GPC], F32, tag="hz", name=f"hzps{k}")
                    nc.tensor.matmul(out=pk[:], lhsT=ident[0:64, :],
                                     rhs=h2r[:, 2 * k, :],
                                     start=True, stop=(k == 6))
                    if k < 6:
                        nc.tensor.matmul(out=pk[:], lhsT=ident_hi[0:64, :],
                                         rhs=h2r[:, 2 * k + 1, :],
                                         start=False, stop=True)
                    nc.vector.tensor_tensor(out=hzT[:, k, :], in0=pk[:],
                                            in1=dinvhz[:, k, :],
                                            op=mybir.AluOpType.mult)
                nc.scalar.activation(out=hzT[:].rearrange("p k g -> p (k g)"),
                                     in_=hzT[:].rearrange("p k g -> p (k g)"),
                                     func=mybir.ActivationFunctionType.Relu,
                                     bias=b2hz[:, 0:1], scale=1.0)
                for m in range(GPC // 128):
                    pf = psF.tile([128, ACT], F32, tag="fin")
                    for k in range(6):
                        nc.tensor.matmul(
                            out=pf[:], lhsT=hzT[:, k, 128 * m:128 * (m + 1)],
                            rhs=WoutSB[:, k, :], start=(k == 0), stop=False)
                    nc.tensor.matmul(
                        out=pf[:], lhsT=hzT[0:64, 6, 128 * m:128 * (m + 1)],
                        rhs=WoutSB[0:64, 6, :], start=False, stop=True)
                    nc.vector.tensor_tensor(out=pf[:], in0=pf[:],
                                            in1=boutrep[:],
                                            op=mybir.AluOpType.add)
                    mx = wk4.tile([128, 1], F32, tag="mx")
                    nc.vector.tensor_reduce(out=mx[:], in_=pf[:],
                                            axis=mybir.AxisListType.X,
                                            op=mybir.AluOpType.max)
                    nmx = wk4.tile([128, 1], F32, tag="nmx")
                    nc.vector.tensor_scalar_mul(nmx[:], mx[:], -1.0)
                    esb = wk4.tile([128, ACT], F32, tag="esb")
                    nc.scalar.activation(out=esb[:], in_=pf[:],
                                         func=mybir.ActivationFunctionType.Exp,
                                         bias=nmx[:, 0:1], scale=1.0)
                    ssum = wk4.tile([128, 1], F32, tag="ssum")
                    nc.vector.tensor_reduce(out=ssum[:], in_=esb[:],
                                            axis=mybir.AxisListType.X,
                                            op=mybir.AluOpType.add)
                    rcp = wk4.tile([128, 1], F32, tag="rcp")
                    nc.vector.reciprocal(out=rcp[:], in_=ssum[:])
                    osb = wk4.tile([128, ACT], F32, tag="osb")
                    nc.vector.tensor_scalar_mul(osb[:], esb[:], rcp[:, 0:1])
                    nc.sync.dma_start(out=d_out[128 * m:128 * (m + 1), :],
                                      in_=osb[:])

    nc.compile()
    return nc


# ---------------------------------------------------------------- entry

_CACHE = {}


def _get(x, ei):
    key = hashlib.sha1(ei.tobytes()).hexdigest()
    if key not in _CACHE:
        meta = _prep(ei)
        nc = _build(meta["L"], meta["calls"], meta["L2"], meta["calls2"])
        _CACHE[key] = (meta, nc)
    return _CACHE[key]


def _in_maps(meta, x, W1, b1, W2, b2, Wout, bout):
    b2t = np.tile(np.asarray(b2, np.float32).reshape(H2), 2).reshape(128, 1)
    maps = []
    for r in range(N_CORES):
        maps.append({
            "xfull": x,
            "xloc": np.ascontiguousarray(x[r * NL:(r + 1) * NL, :]),
            "idx": np.ascontiguousarray(meta["idx_sb"][r]),
            "dstv": np.ascontiguousarray(meta["dstv_sb"][r]),
            "coef": np.ascontiguousarray(meta["coef_sb"][r]),
            "idx2": np.ascontiguousarray(meta["idx2_sb"][r]),
            "dstv2": np.ascontiguousarray(meta["dstv2_sb"][r]),
            "dinvt": np.ascontiguousarray(meta["dinv_tiles"][r]),
            "dinv2t": np.ascontiguousarray(meta["dinv2_tiles"][r]),
            "dinvhz": np.ascontiguousarray(
                meta["dinv_hz"][r].reshape(128, 7 * GPC)),
            "W1": np.ascontiguousarray(W1, np.float32),
            "b1": np.ascontiguousarray(b1, np.float32).reshape(H1, 1),
            "W2": np.ascontiguousarray(W2, np.float32),
            "b2hz": b2t,
            "Wout": np.ascontiguousarray(Wout, np.float32),
            "bout": np.ascontiguousarray(bout, np.float32).reshape(1, ACT),
        })
    return maps


def kernel(x, ei, W1, b1, W2, b2, Wout, bout, _trace=False):
    x = np.ascontiguousarray(x, np.float32)
    ei = np.ascontiguousarray(ei, np.int32)
    meta, nc = _get(x, ei)
    maps = _in_maps(meta, x, W1, b1, W2, b2, Wout, bout)
    res = bass_utils.run_bass_kernel_spmd(
        nc, maps, core_ids=list(range(N_CORES)), trace=_trace)
    out = np.concatenate([res.results[r]["out"] for r in range(N_CORES)],
                         axis=0).astype(np.float32)
    if _trace:
        return out, res.exec_time_ns
    return out


def install_profile_hook():
    import types
    sys.path.insert(0, "/root/.axon_site")
    import trn_agent_boot.trn_boot as _tb
    import antenv
    if "antenv.axon_hooks" not in sys.modules:
        _mod = types.ModuleType("antenv.axon_hooks")
        _h = [None]
        _mod.set_axon_ntff_profile_hook = lambda h: _h.__setitem__(0, h)
        _mod.get_axon_ntff_profile_hook = lambda: _h[0]
        sys.modules["antenv.axon_hooks"] = _mod
        antenv.axon_hooks = _mod
        _mod.set_axon_ntff_profile_hook(
            _tb._ntff_profile_via_ctypes("/opt/axon/libaxon_pjrt.so"))


# revision 15
# speedup vs baseline: 1.6020x; 1.0538x over previous
"""Trainium2 Bass kernel for nn_ActorNetwork (2-layer GCN + actor head).

Self-contained: hardcodes all shapes/sharding (8 NeuronCores).

Strategy:
  - Shard dst nodes (= graphs) contiguously across 8 cores (10240 nodes =
    256 graphs per core).
  - Edges are random over the full node set; each layer gathers source rows
    with gpsimd dma_gather (edges sorted by (src-chunk, dst-tile) on host),
    aggregates per 128-dst tile with one-hot matmuls on TensorE (bf16).
  - Self-loops bypass the gather (sequential stream + PE transpose).
  - Layer 2 is PRUNED to host destinations only (the actor head reads just
    the first 13 of every 40 nodes), cutting L2 edges 40/13 ~ 3x.
  - Between layers: AllGather of the dinv-prescaled h1@W2 ("m2s"), stored
    as bf16 padded to 128 cols (256B rows) so gathered rows feed the PE
    scatter matmuls directly with no per-edge cast.
  - Head: host rows are static; dst-side dinv/bias/relu applied on the
    selected slots only; f32 GEMM + softmax.
"""
import sys
import hashlib

sys.path.insert(0, "/opt/trn_rl_repo")

import numpy as np
import ml_dtypes
from contextlib import ExitStack

from concourse import bass, mybir, tile, bass_utils, bacc
from concourse.masks import make_identity

F32 = mybir.dt.float32
BF16 = mybir.dt.bfloat16
I16 = mybir.dt.int16
I32 = mybir.dt.int32

N_CORES = 8
N = 81920
NL = N // N_CORES          # 10240 nodes per core
IN_DIM = 128
H1 = 256
H2 = 64
GRAPH = 40
NH = 13
ACT = 145
TILES = NL // 128          # 80 dst tiles per core (layer 1)
GPC = NL // GRAPH          # 256 graphs per core
NHL = GPC * NH             # 3328 host nodes per core
T2 = NHL // 128            # 26 dst tiles per core (layer 2, hosts only)
CHUNK = 32768
CHUNKS = [(0, 32768), (32768, 32768), (65536, 16384)]
NCH = 3
CALL_G = 16                # groups (of 128 idxs) per dma_gather call
SENT = 300.0               # sentinel dst value for padding slots


# ---------------------------------------------------------------- host prep

def _plan(core, c_of, t_of, idxl, dloc, coefv, n_tiles):
    """Build the per-core slot array + call/matmul-event schedule for one
    edge set (edges described by per-edge core/chunk/tile/local-idx/dst-loc).
    Slot structure is shared across cores (SPMD): per-(chunk,tile) segment
    sizes are the max over cores."""
    counts = np.zeros((N_CORES, NCH, n_tiles), np.int64)
    np.add.at(counts, (core, c_of, t_of), 1)
    Ncm = counts.max(axis=0)                        # [3, n_tiles]
    seg_off = np.zeros((NCH, n_tiles), np.int64)    # global slot offsets
    chunk_base = np.zeros(NCH + 1, np.int64)
    calls = []   # (chunk, slot0_global, n_g, events)
    off = 0
    for c in range(NCH):
        chunk_base[c] = off
        for t in range(n_tiles):
            seg_off[c, t] = off
            off += int(Ncm[c, t])
        off = ((off - chunk_base[c] + 127) // 128 + 0) * 128 + chunk_base[c] \
            if (off - chunk_base[c]) % 128 else off
    chunk_base[NCH] = off
    L = int(off)

    # per-chunk group structure + matmul/drain events
    for c in range(NCH):
        base = int(chunk_base[c])
        S = int(chunk_base[c + 1] - base)
        ngroups = S // 128
        # tile of each group's first slot
        def tile_of(slot):
            # slot is chunk-local
            j = np.searchsorted(seg_off[c] - base, slot, side="right") - 1
            j = max(0, min(n_tiles - 1, int(j)))
            if slot >= int(seg_off[c, j] - base) + int(Ncm[c, j]):
                return -1          # chunk-tail pad region
            return j
        tg = [tile_of(128 * g) for g in range(ngroups)]
        for g in range(ngroups):
            if tg[g] == -1:
                tg[g] = n_tiles - 1  # tail pads: harmless window
        # first/last group of each tile's segment
        g_a = [(int(seg_off[c, t] - base)) // 128 for t in range(n_tiles)]
        g_b = [(int(seg_off[c, t] - base) + int(Ncm[c, t]) - 1) // 128
               for t in range(n_tiles)]
        # build matmul event list in group order
        events_all = []
        for g in range(ngroups):
            t0 = tg[g]
            seg_end = int(seg_off[c, t0] - base) + int(Ncm[c, t0])
            spans = (t0 + 1 < n_tiles) and (128 * (g + 1) > seg_end)
            if spans:
                assert 128 * (g + 1) <= seg_end + int(Ncm[c, t0 + 1]), \
                    "group spans >2 tiles"
            evs = [(g, 0, t0, g == g_a[t0], g == g_b[t0])]
            if spans:
                t1 = t0 + 1
                evs.append((g, 1, t1, g == g_a[t1], g == g_b[t1]))
            events_all.append(evs)
        # slice into calls
        gi = 0
        while gi < ngroups:
            n = min(CALL_G, ngroups - gi)
            evs = []
            for g in range(gi, gi + n):
                for (gg, half, t, st, sp) in events_all[g]:
                    evs.append((gg - gi, half, t, st, sp))
            calls.append((c, base + 128 * gi, n, evs))
            gi += n

    idx_all = np.zeros((N_CORES, L), np.int16)
    dstv_all = np.full((N_CORES, L), SENT, np.float32)
    coef_all = np.zeros((N_CORES, L), np.float32)
    # group tile map per global slot (for relative dstv)
    tg_of_slot = np.full(L, -1, np.int64)
    for c in range(NCH):
        base = int(chunk_base[c])
        S = int(chunk_base[c + 1] - base)
        for g in range(S // 128):
            j = np.searchsorted(seg_off[c] - base, 128 * g, side="right") - 1
            j = max(0, min(n_tiles - 1, int(j)))
            if 128 * g >= int(seg_off[c, j] - base) + int(Ncm[c, j]):
                j = n_tiles - 1
            tg_of_slot[base + 128 * g: base + 128 * (g + 1)] = j
    for r in range(N_CORES):
        m = core == r
        sc, st = c_of[m], t_of[m]
        si, sd = idxl[m], dloc[m]
        scf = coefv[m] if coefv is not None else None
        order = np.lexsort((st, sc))
        sc, st = sc[order], st[order]
        si, sd = si[order], sd[order]
        key = sc * n_tiles + st
        change = np.r_[True, key[1:] != key[:-1]]
        starts = np.flatnonzero(change)
        runid = np.cumsum(change) - 1
        within = np.arange(len(key)) - starts[runid]
        base_run = seg_off[sc[starts], st[starts]]
        pos = base_run[runid] + within
        idx_all[r, pos] = si.astype(np.int16)
        dstv_all[r, pos] = sd + 128.0 * (st - tg_of_slot[pos])
        if scf is not None:
            coef_all[r, pos] = scf[order]

    idx_sb = np.stack([
        np.tile(idx_all[r].reshape(-1, 16).T, (8, 1)) for r in range(N_CORES)
    ])                                               # [8, 128, L/16]
    dstv_sb = np.stack([
        dstv_all[r].reshape(-1, 128).T for r in range(N_CORES)
    ]).astype(ml_dtypes.bfloat16)                    # [8, 128, L/128]
    coef_sb = np.stack([
        coef_all[r].reshape(-1, 128).T for r in range(N_CORES)
    ])                                               # [8, 128, L/128]
    return L, calls, idx_sb, dstv_sb, coef_sb


def _prep(ei):
    src = ei[0].astype(np.int64)
    dst = ei[1].astype(np.int64)
    deg = np.bincount(dst, minlength=N).astype(np.float64) + 1.0
    dinv = (1.0 / np.sqrt(deg)).astype(np.float32)
    coef = (dinv[src] * dinv[dst]).astype(np.float32)

    # ---- layer-1 plan: all edges, dst tiles over all local nodes
    core = dst // NL
    t_of = (dst % NL) // 128
    c_of = src // CHUNK
    idxl = src % CHUNK
    dloc = (dst % 128).astype(np.float32)
    L, calls, idx_sb, dstv_sb, coef_sb = _plan(
        core, c_of, t_of, idxl, dloc, coef, TILES)

    # ---- layer-2 plan: host-dst edges only, dst tiles over host slots
    hmask = (dst % GRAPH) < NH
    src2, dst2 = src[hmask], dst[hmask]
    core2 = dst2 // NL
    hostloc = (dst2 % NL) // GRAPH * NH + dst2 % GRAPH
    t2_of = hostloc // 128
    c2_of = src2 // CHUNK
    idxl2 = src2 % CHUNK
    dloc2 = (hostloc % 128).astype(np.float32)
    L2, calls2, idx2_sb, dstv2_sb, _ = _plan(
        core2, c2_of, t2_of, idxl2, dloc2, None, T2)

    dinv_l = dinv.reshape(N_CORES, NL)
    dinv_tiles = np.ascontiguousarray(
        dinv_l.reshape(N_CORES, TILES, 128).transpose(0, 2, 1))   # [8,128,80]
    dinv2_tiles = (dinv_tiles ** 2).astype(np.float32)

    # per-slot dst dinv for the head: hzT[p, k, g] -> host h=2k+(p>=64),
    # feat=p%64, local node g*40+h
    dinv_hz = np.zeros((N_CORES, 128, 7, GPC), np.float32)
    for k in range(7):
        for half in range(2):
            h = 2 * k + half
            if h >= NH:
                continue
            nodes = np.arange(GPC) * GRAPH + h
            dinv_hz[:, 64 * half:64 * (half + 1), k, :] = \
                dinv_l[:, nodes][:, None, :]

    return dict(L=L, calls=calls, idx_sb=idx_sb, dstv_sb=dstv_sb,
                coef_sb=coef_sb, L2=L2, calls2=calls2, idx2_sb=idx2_sb,
                dstv2_sb=dstv2_sb, dinv_tiles=dinv_tiles,
                dinv2_tiles=dinv2_tiles, dinv_hz=dinv_hz)


# ---------------------------------------------------------------- builder

def _edge_pass(nc, wk, psA, calls, src_dram, elem, idxt, dstvt, coeft,
               iota_bf, agg, selfT, l2, self_lhs=None, on_close=None,
               prep_n=0, dma_sem=None, gat_bufs=3):
    """Shared edge-aggregation pass for both layers (256-dst windows).

    l2=False: gather f32 rows, scale by per-edge coef -> bf16 lhsT.
    l2=True: gather bf16 rows (padded to `elem`), first 64 cols are the
    payload and feed the PE directly (no per-edge vector op).
    self_lhs=(xb, dg): fold the self-loop term into each tile's psum as an
    extra matmul (xb[:,t,:]^T @ diag) when the tile opens in chunk 0;
    otherwise the self term is added from `selfT` at chunk-0 close."""
    open_ps = {}
    gdt = BF16 if l2 else F32
    for (c, slot0, n_g, events) in calls:
        rows0, nrows = CHUNKS[c]
        gat = wk.tile([128, CALL_G, elem], gdt, tag="gat", bufs=3)
        nc.gpsimd.dma_gather(
            out_ap=gat[:, 0:n_g, :],
            in_ap=src_dram[rows0:rows0 + nrows, :],
            idxs_ap=idxt[:, slot0 // 16: slot0 // 16 + n_g * 8],
            num_idxs=n_g * 128, num_idxs_reg=n_g * 128,
            elem_size=elem, single_packet=False)
        s0 = slot0 // 128
        if l2:
            gatb = gat
        else:
            gatb = wk.tile([128, CALL_G, elem], BF16, tag="gatb", bufs=3)
            nc.vector.tensor_tensor(
                out=gatb[:, 0:n_g, :], in0=gat[:, 0:n_g, :],
                in1=coeft[:, s0:s0 + n_g].unsqueeze(2).to_broadcast(
                    (128, n_g, elem)),
                op=mybir.AluOpType.mult)
        oh = wk.tile([128, CALL_G, 256], BF16, tag="oh", bufs=3)
        nc.vector.tensor_tensor(
            out=oh[:, 0:n_g, :],
            in0=dstvt[:, s0:s0 + n_g].unsqueeze(2).to_broadcast((128, n_g, 256)),
            in1=iota_bf[:].unsqueeze(1).to_broadcast((128, n_g, 256)),
            op=mybir.AluOpType.is_equal)
        M = 64 if l2 else 128
        for (g, half, t, first, last) in events:
            if first:
                open_ps[t] = psA.tile([M, 128], F32, tag="agg",
                                      name=f"aggps_c{c}_t{t}")
                start = True
                if c == 0 and self_lhs is not None:
                    xb, dg = self_lhs
                    nc.tensor.matmul(out=open_ps[t][:], lhsT=xb[:, t, :],
                                     rhs=dg[:, t, :], start=True, stop=False)
                    start = False
            else:
                start = False
            ps = open_ps[t]
            lhsT = gatb[:, g, 0:64] if l2 else gatb[:, g, :]
            nc.tensor.matmul(out=ps[:], lhsT=lhsT,
                             rhs=oh[:, g, 128 * half:128 * (half + 1)],
                             start=start, stop=last)
            if last:
                sl = slice(128 * t, 128 * (t + 1))
                if c == 0:
                    if self_lhs is not None:
                        nc.vector.tensor_copy(out=agg[:, sl], in_=ps[:])
                    else:
                        nc.vector.tensor_tensor(out=agg[:, sl], in0=ps[:],
                                                in1=selfT[:, sl],
                                                op=mybir.AluOpType.add)
                else:
                    nc.vector.tensor_tensor(out=agg[:, sl], in0=agg[:, sl],
                                            in1=ps[:], op=mybir.AluOpType.add)
                del open_ps[t]
                if on_close is not None:
                    on_close(t)


def _build(L, calls, L2, calls2):
    nc = bacc.Bacc("TRN2", target_bir_lowering=False, debug=False,
                   num_devices=N_CORES)
    d_xfull = nc.dram_tensor("xfull", [N, IN_DIM], F32, kind="ExternalInput")
    # local x pre-permuted on host to [p, tile, feat] so the casting DMA to
    # SBUF is one contiguous descriptor per partition
    d_xloc = nc.dram_tensor("xloc", [128, TILES * IN_DIM], F32,
                            kind="ExternalInput")
    d_idx = nc.dram_tensor("idx", [128, L // 16], I16, kind="ExternalInput")
    d_dstv = nc.dram_tensor("dstv", [128, L // 128], BF16, kind="ExternalInput")
    d_coef = nc.dram_tensor("coef", [128, L // 128], F32, kind="ExternalInput")
    d_idx2 = nc.dram_tensor("idx2", [128, L2 // 16], I16, kind="ExternalInput")
    d_dstv2 = nc.dram_tensor("dstv2", [128, L2 // 128], BF16,
                             kind="ExternalInput")
    d_dinvt = nc.dram_tensor("dinvt", [128, TILES], F32, kind="ExternalInput")
    d_dinv2t = nc.dram_tensor("dinv2t", [128, TILES], F32, kind="ExternalInput")
    d_dinvhz = nc.dram_tensor("dinvhz", [128, 7 * GPC], F32, kind="ExternalInput")
    d_W1 = nc.dram_tensor("W1", [IN_DIM, H1], F32, kind="ExternalInput")
    d_b1 = nc.dram_tensor("b1", [H1, 1], F32, kind="ExternalInput")
    d_W2 = nc.dram_tensor("W2", [H1, H2], F32, kind="ExternalInput")
    d_b2hz = nc.dram_tensor("b2hz", [128, 1], F32, kind="ExternalInput")
    d_Wout = nc.dram_tensor("Wout", [NH * H2, ACT], F32, kind="ExternalInput")
    d_bout = nc.dram_tensor("bout", [1, ACT], F32, kind="ExternalInput")
    d_out = nc.dram_tensor("out", [GPC, ACT], F32, kind="ExternalOutput")

    with tile.TileContext(nc) as tc, ExitStack() as top:
        perm = top.enter_context(tc.tile_pool(name="perm", bufs=1))
        dram = top.enter_context(tc.tile_pool(name="dram", bufs=1, space="DRAM"))

        # ---- persistent tiles
        idxt = perm.tile([128, L // 16], I16)
        nc.sync.dma_start(out=idxt[:], in_=d_idx[:])
        dstvt = perm.tile([128, L // 128], BF16)
        nc.sync.dma_start(out=dstvt[:], in_=d_dstv[:])
        coeft = perm.tile([128, L // 128], F32)
        nc.sync.dma_start(out=coeft[:], in_=d_coef[:])
        idxt2 = perm.tile([128, L2 // 16], I16)
        nc.sync.dma_start(out=idxt2[:], in_=d_idx2[:])
        dstvt2 = perm.tile([128, L2 // 128], BF16)
        nc.sync.dma_start(out=dstvt2[:], in_=d_dstv2[:])
        dinvt = perm.tile([128, TILES], F32)
        nc.sync.dma_start(out=dinvt[:], in_=d_dinvt[:])
        dinv2t = perm.tile([128, TILES], F32)
        nc.sync.dma_start(out=dinv2t[:], in_=d_dinv2t[:])
        b1sb = perm.tile([128, 2], F32)
        nc.sync.dma_start(out=b1sb[:, 0:1], in_=d_b1[0:128, :])
        nc.sync.dma_start(out=b1sb[:, 1:2], in_=d_b1[128:256, :])
        b2hz = perm.tile([128, 1], F32)
        nc.sync.dma_start(out=b2hz[:], in_=d_b2hz[:])
        WoutSB = perm.tile([128, 7, ACT], F32)
        for k in range(6):
            nc.sync.dma_start(out=WoutSB[:, k, :],
                              in_=d_Wout[128 * k:128 * (k + 1), :])
        nc.sync.dma_start(out=WoutSB[0:64, 6, :], in_=d_Wout[768:832, :])
        boutrep = perm.tile([128, ACT], F32)
        nc.sync.dma_start(out=boutrep[:], in_=d_bout[:].to_broadcast((128, ACT)))
        dinvhz = perm.tile([128, 7, GPC], F32)
        nc.sync.dma_start(out=dinvhz[:].rearrange("p k g -> p (k g)"),
                          in_=d_dinvhz[:])

        # identities built on DVE from iotas (keeps gpsimd free for gathers)
        iota_i = perm.tile([128, 256], I32)
        nc.gpsimd.iota(iota_i[:], pattern=[[1, 256]], base=0,
                       channel_multiplier=0)
        iota_bf = perm.tile([128, 256], BF16)
        nc.vector.tensor_copy(out=iota_bf[:], in_=iota_i[:])
        iota_f = perm.tile([128, 128], F32)
        nc.vector.tensor_copy(out=iota_f[:], in_=iota_i[:, 0:128])
        pidx = perm.tile([128, 1], I32)
        nc.gpsimd.iota(pidx[:], pattern=[[1, 1]], base=0, channel_multiplier=1)
        pidxf = perm.tile([128, 1], F32)
        nc.vector.tensor_copy(out=pidxf[:], in_=pidx[:])
        ident = perm.tile([128, 128], F32)
        nc.vector.tensor_tensor(out=ident[:],
                                in0=pidxf[:].to_broadcast((128, 128)),
                                in1=iota_f[:], op=mybir.AluOpType.is_equal)
        identb = perm.tile([128, 128], BF16)
        nc.vector.tensor_copy(out=identb[:], in_=ident[:])
        ioc = perm.tile([128, 1], I32)
        nc.gpsimd.iota(ioc[:], pattern=[[1, 1]], base=64, channel_multiplier=1)
        iocf = perm.tile([128, 1], F32)
        nc.vector.tensor_copy(out=iocf[:], in_=ioc[:])
        # ident_hi[p, j] = 1 if j == p + 64 (used to shift rows up by 64)
        ident_hi = perm.tile([128, 128], F32)
        nc.vector.tensor_tensor(out=ident_hi[:],
                                in0=iocf[:].to_broadcast((128, 128)),
                                in1=iota_f[:], op=mybir.AluOpType.is_equal)

        m2sl = dram.tile([NL, 128], BF16)
        m2sf = dram.tile([N, 128], BF16, addr_space="Shared")

        c0_calls = [cl for cl in calls if cl[0] == 0]
        c1_calls = [cl for cl in calls if cl[0] == 1]
        c2_calls = [cl for cl in calls if cl[0] == 2]

        # =========================== Layer 1 ===========================
        with ExitStack() as ph1:
            mid1 = ph1.enter_context(tc.tile_pool(name="mid1", bufs=1))
            wk1 = ph1.enter_context(tc.tile_pool(name="wk1", bufs=2))
            psA = ph1.enter_context(tc.tile_pool(name="psA", bufs=4,
                                                 space="PSUM"))
            agg1 = mid1.tile([128, NL], BF16)

            es0 = ExitStack()
            mid0 = es0.enter_context(tc.tile_pool(name="mid0", bufs=1))
            # local x as bf16 [p, tile, feat] via one casting DMA; self-loop
            # contribution becomes a per-tile transpose-matmul against a
            # dinv^2-scaled diagonal.
            xbts = mid0.tile([128, TILES, 128], BF16)
            nc.gpsimd.dma_start(out=xbts[:].rearrange("p t f -> p (t f)"),
                                in_=d_xloc[:])
            diags = mid0.tile([128, TILES, 128], BF16)
            nc.vector.tensor_tensor(
                out=diags[:],
                in0=identb[:].unsqueeze(1).to_broadcast((128, TILES, 128)),
                in1=dinv2t[:].unsqueeze(2).to_broadcast((128, TILES, 128)),
                op=mybir.AluOpType.mult)

            _edge_pass(nc, wk1, psA, c0_calls, d_xfull, IN_DIM, idxt, dstvt,
                       coeft, iota_bf, agg1, None, l2=False,
                       self_lhs=(xbts, diags))
            es0.close()
            # weight casts on gpsimd can go here: after chunk-0 gathers,
            # well before GEMM1/GEMM2 need them
            W1sb = perm.tile([128, H1], BF16)
            nc.gpsimd.dma_start(out=W1sb[:], in_=d_W1[:])
            W2sb = perm.tile([128, 2, H2], BF16)
            nc.gpsimd.dma_start(out=W2sb[:, 0, :], in_=d_W2[0:128, :])
            nc.gpsimd.dma_start(out=W2sb[:, 1, :], in_=d_W2[128:256, :])

            _edge_pass(nc, wk1, psA, c1_calls, d_xfull, IN_DIM, idxt, dstvt,
                       coeft, iota_bf, agg1, None, l2=False)

            # chunk 2: as tiles finalize, stream GEMM1 -> GEMM2 -> m2s stage
            # -> m2sl writeback behind the remaining gathers
            midS = ph1.enter_context(tc.tile_pool(name="midS", bufs=1))
            h1T = midS.tile([128, 2, NL], BF16)
            stage = midS.tile([128, TILES, 128], BF16)
            m2sTs = perm.tile([64, NL], BF16)
            nc.vector.memset(stage[:, :, 64:128], 0.0)
            psG1 = ph1.enter_context(tc.tile_pool(name="psG1", bufs=1,
                                                  space="PSUM"))
            psG2 = ph1.enter_context(tc.tile_pool(name="psG2", bufs=1,
                                                  space="PSUM"))
            psT2 = ph1.enter_context(tc.tile_pool(name="psT2", bufs=1,
                                                  space="PSUM"))
            g1_next = [0]

            def flush_blocks(t_closed):
                while (g1_next[0] + 1) * 4 <= t_closed + 1:
                    nb = g1_next[0]
                    sl512 = slice(512 * nb, 512 * (nb + 1))
                    for m in range(2):
                        pg = psG1.tile([128, 512], F32, tag="g1")
                        nc.tensor.matmul(
                            out=pg[:], lhsT=W1sb[:, 128 * m:128 * (m + 1)],
                            rhs=agg1[:, sl512], start=True, stop=True)
                        nc.scalar.activation(
                            out=h1T[:, m, sl512], in_=pg[:],
                            func=mybir.ActivationFunctionType.Relu,
                            bias=b1sb[:, m:m + 1], scale=1.0)
                    for tt in range(4 * nb, 4 * nb + 4):
                        pg2 = psG2.tile([128, H2], F32, tag="g2")
                        for m in range(2):
                            nc.tensor.matmul(
                                out=pg2[:],
                                lhsT=h1T[:, m, 128 * tt:128 * (tt + 1)],
                                rhs=W2sb[:, m, :], start=(m == 0),
                                stop=(m == 1))
                        nc.vector.tensor_scalar_mul(stage[:, tt, 0:64],
                                                    pg2[:], dinvt[:, tt:tt + 1])
                        pt = psT2.tile([64, 128], BF16, tag="tr2")
                        nc.tensor.transpose(out=pt[:], in_=stage[:, tt, 0:64],
                                            identity=identb[:])
                        nc.scalar.activation(
                            out=m2sTs[:, 128 * tt:128 * (tt + 1)], in_=pt[:],
                            func=mybir.ActivationFunctionType.Copy)
                    nc.sync.dma_start(
                        out=m2sl[512 * nb:512 * (nb + 1), :].rearrange(
                            "(t p) f -> p t f", p=128),
                        in_=stage[:, 4 * nb:4 * nb + 4, :])
                    g1_next[0] += 1

            _edge_pass(nc, wk1, psA, c2_calls, d_xfull, IN_DIM, idxt, dstvt,
                       coeft, iota_bf, agg1, None, l2=False,
                       on_close=flush_blocks)
            flush_blocks(TILES - 1)

        # ==================== m2s AllGather + Layer 2 ==================
        with ExitStack() as ph2:
            midA = ph2.enter_context(tc.tile_pool(name="midA", bufs=1))
            selfTh = midA.tile([64, NHL], BF16)
            agg2 = midA.tile([64, NHL], F32)
            # self-loop payload at host slots: hostloc g*13+h <- node g*40+h
            sTh = selfTh[:].rearrange("p (g q) -> p q g", q=NH)
            mTs = m2sTs[:].rearrange("p (g q) -> p q g", q=GRAPH)
            for h in range(NH):
                nc.vector.tensor_copy(out=sTh[:, h, :], in_=mTs[:, h, :])
            nc.gpsimd.collective_compute(
                "AllGather", mybir.AluOpType.bypass,
                replica_groups=[list(range(N_CORES))],
                ins=[m2sl[:].opt()], outs=[m2sf[:].opt()])

            # ========================= Layer 2 =========================
            with ExitStack() as ph3:
                wk2 = ph3.enter_context(tc.tile_pool(name="wk2", bufs=2))
                psA2 = ph3.enter_context(tc.tile_pool(name="psA2", bufs=4,
                                                      space="PSUM"))
                _edge_pass(nc, wk2, psA2, calls2, m2sf, 128, idxt2, dstvt2,
                           None, iota_bf, agg2, selfTh, l2=True)

            # ===================== actor head ==========================
            with ExitStack() as ph4:
                mid4 = ph4.enter_context(tc.tile_pool(name="mid4", bufs=1))
                wk4 = ph4.enter_context(tc.tile_pool(name="wk4", bufs=2))
                psF = ph4.enter_context(tc.tile_pool(name="psF", bufs=2,
                                                     space="PSUM"))
                hzT = mid4.tile([128, 7, GPC], F32)
                h2r = agg2[:].rearrange("p (g q) -> p q g", q=NH)
                for k in range(7):
                    pk = psF.tile([128, GPC], F32, tag="hz", name=f"hzps{k}")
                    nc.tensor.matmul(out=pk[:], lhsT=ident[0:64, :],
                                     rhs=h2r[:, 2 * k, :],
                                     start=True, stop=(k == 6))
                    if k < 6:
                        nc.tensor.matmul(out=pk[:], lhsT=ident_hi[0:64, :],
                                         rhs=h2r[:, 2 * k + 1, :],
                                         start=False, stop=True)
                    nc.vector.tensor_tensor(out=hzT[:, k, :], in0=pk[:],
                                            in1=dinvhz[:, k, :],
                                            op=mybir.AluOpType.mult)
                nc.scalar.activation(out=hzT[:].rearrange("p k g -> p (k g)"),
                                     in_=hzT[:].rearrange("p k g -> p (k g)"),
                                     func=mybir.ActivationFunctionType.Relu,
                                     bias=b2hz[:, 0:1], scale=1.0)
                for m in range(GPC // 128):
                    pf = psF.tile([128, ACT], F32, tag="fin")
                    for k in range(6):
                        nc.tensor.matmul(
                            out=pf[:], lhsT=hzT[:, k, 128 * m:128 * (m + 1)],
                            rhs=WoutSB[:, k, :], start=(k == 0), stop=False)
                    nc.tensor.matmul(
                        out=pf[:], lhsT=hzT[0:64, 6, 128 * m:128 * (m + 1)],
                        rhs=WoutSB[0:64, 6, :], start=False, stop=True)
                    nc.vector.tensor_tensor(out=pf[:], in0=pf[:],
                                            in1=boutrep[:],
                                            op=mybir.AluOpType.add)
                    mx = wk4.tile([128, 1], F32, tag="mx")
                    nc.vector.tensor_reduce(out=mx[:], in_=pf[:],
                                            axis=mybir.AxisListType.X,
                                            op=mybir.AluOpType.max)
                    nmx = wk4.tile([128, 1], F32, tag="nmx")
                    nc.vector.tensor_scalar_mul(nmx[:], mx[:], -1.0)
                    esb = wk4.tile([128, ACT], F32, tag="esb")
                    nc.scalar.activation(out=esb[:], in_=pf[:],
                                         func=mybir.ActivationFunctionType.Exp,
                                         bias=nmx[:, 0:1], scale=1.0)
                    ssum = wk4.tile([128, 1], F32, tag="ssum")
                    nc.vector.tensor_reduce(out=ssum[:], in_=esb[:],
                                            axis=mybir.AxisListType.X,
                                            op=mybir.AluOpType.add)
                    rcp = wk4.tile([128, 1], F32, tag="rcp")
                    nc.vector.reciprocal(out=rcp[:], in_=ssum[:])
                    osb = wk4.tile([128, ACT], F32, tag="osb")
                    nc.vector.tensor_scalar_mul(osb[:], esb[:], rcp[:, 0:1])
                    nc.sync.dma_start(out=d_out[128 * m:128 * (m + 1), :],
                                      in_=osb[:])

    nc.compile()
    return nc


# ---------------------------------------------------------------- entry

_CACHE = {}


def _get(x, ei):
    key = hashlib.sha1(ei.tobytes()).hexdigest()
    if key not in _CACHE:
        meta = _prep(ei)
        nc = _build(meta["L"], meta["calls"], meta["L2"], meta["calls2"])
        _CACHE[key] = (meta, nc)
    return _CACHE[key]


def _in_maps(meta, x, W1, b1, W2, b2, Wout, bout):
    b2t = np.tile(np.asarray(b2, np.float32).reshape(H2), 2).reshape(128, 1)
    maps = []
    for r in range(N_CORES):
        maps.append({
            "xfull": x,
            "xloc": np.ascontiguousarray(
                x[r * NL:(r + 1) * NL, :].reshape(TILES, 128, IN_DIM)
                .transpose(1, 0, 2).reshape(128, TILES * IN_DIM)),
            "idx": np.ascontiguousarray(meta["idx_sb"][r]),
            "dstv": np.ascontiguousarray(meta["dstv_sb"][r]),
            "coef": np.ascontiguousarray(meta["coef_sb"][r]),
            "idx2": np.ascontiguousarray(meta["idx2_sb"][r]),
            "dstv2": np.ascontiguousarray(meta["dstv2_sb"][r]),
            "dinvt": np.ascontiguousarray(meta["dinv_tiles"][r]),
            "dinv2t": np.ascontiguousarray(meta["dinv2_tiles"][r]),
            "dinvhz": np.ascontiguousarray(
                meta["dinv_hz"][r].reshape(128, 7 * GPC)),
            "W1": np.ascontiguousarray(W1, np.float32),
            "b1": np.ascontiguousarray(b1, np.float32).reshape(H1, 1),
            "W2": np.ascontiguousarray(W2, np.float32),
            "b2hz": b2t,
            "Wout": np.ascontiguousarray(Wout, np.float32),
            "bout": np.ascontiguousarray(bout, np.float32).reshape(1, ACT),
        })
    return maps


def kernel(x, ei, W1, b1, W2, b2, Wout, bout, _trace=False):
    x = np.ascontiguousarray(x, np.float32)
    ei = np.ascontiguousarray(ei, np.int32)
    meta, nc = _get(x, ei)
    maps = _in_maps(meta, x, W1, b1, W2, b2, Wout, bout)
    res = bass_utils.run_bass_kernel_spmd(
        nc, maps, core_ids=list(range(N_CORES)), trace=_trace)
    out = np.concatenate([res.results[r]["out"] for r in range(N_CORES)],
                         axis=0).astype(np.float32)
    if _trace:
        return out, res.exec_time_ns
    return out


def install_profile_hook():
    import types
    sys.path.insert(0, "/root/.axon_site")
    import trn_agent_boot.trn_boot as _tb
    import antenv
    if "antenv.axon_hooks" not in sys.modules:
        _mod = types.ModuleType("antenv.axon_hooks")
        _h = [None]
        _mod.set_axon_ntff_profile_hook = lambda h: _h.__setitem__(0, h)
        _mod.get_axon_ntff_profile_hook = lambda: _h[0]
        sys.modules["antenv.axon_hooks"] = _mod
        antenv.axon_hooks = _mod
        _mod.set_axon_ntff_profile_hook(
            _tb._ntff_profile_via_ctypes("/opt/axon/libaxon_pjrt.so"))
